# revision 1
# baseline (speedup 1.0000x reference)
"""Trainium2 Bass kernel: attention layer (B=4, S=2048, D=1024), 8 NeuronCores.

Sharding: data-parallel over (batch, query-half) -> 8 shards. Each core
computes one batch's half of the queries against that batch's full key/value
(tensor-parallel K/V splits were measured and rejected: a pairwise 4MB
AllGather costs >150us on this part, far more than the 60us of matmuls
it would save).

Per-core dataflow (all transpose-free; host pre-transposes inputs):
  QT[e,q]   = Wq projection of queries (fp32r matmuls; stays resident)
  KT chunks = Wk projection, fused with the score phase: each [e,512k]
              drain chunk is consumed as score-matmul lhsT straight from
              SBUF, so KT is never materialized or spilled
  ST[k,q]   = scores, k on partitions (fp32r, fp32 PSUM); spilled to DRAM
              except the last two k-tiles; row-max tracked on the fly by
              a DVE max chain
  softmax along k (the partition dim): 7-step DMA-shift partition-halving
    max -> m[1,q]; broadcast to [128,q] via DRAM bounce + zero-partition-
    stride reload; E = exp(ST - m) in bf16 (ST reloads land in the freed
    QT slots; subtract runs in place); l accumulated on DVE as E appears
  V[k,e]    = value projection (fp32r from the resident key tiles),
              emitted after the scores so the PE stays busy through the
              softmax reductions; spills to DRAM, reloads into freed SBUF
  O[q,e]    = (E.T @ V) matmuls (bf16); the l row-sum ones-matmuls, the
              PE-transpose of 1/l into [q,1] layout, and the scaled
              stores are threaded into the middle of the PV loop
Engine budget at ~330us: PE ~281us busy (84%), DVE ~137, ACT ~49,
gpsimd ~62, sync ~36; MFU ~72%.

float32r matmul measured on silicon: ~1 cycle/row at free-dim 512 (vs
fp32's 4) with ~1.5e-4 relative precision -- enough for the unscaled
(near-one-hot, logit std ~34) softmax here; bf16 scores would flip
argmax rows and fail (measured 2.5e-2). bf16 is fine for E and the PV
contraction. End-to-end rel err vs the fp32 reference: 2.3e-3.
"""

import numpy as np
import ml_dtypes
from contextlib import ExitStack

import concourse.bass as bass
import concourse.tile as tile
from concourse import bacc, mybir
from concourse.bass import ts
from concourse.bass_utils import run_bass_kernel_spmd

B, S, D = 4, 2048, 1024
N_CORES = 8
SQ = S // 2            # 1024 query rows per core
P = 128                # partitions
NE = D // P            # 8 e-tiles
ND = D // P            # 8 d-tiles
NK = S // P            # 16 k-tiles
NQC = SQ // P          # 8 q-chunks
F32R = mybir.dt.float32r
F32 = mybir.dt.float32
BF16 = mybir.dt.bfloat16

_NC_CACHE = {}


def _build():
    if "nc" in _NC_CACHE:
        return _NC_CACHE["nc"]
    nc = bacc.Bacc("TRN2", target_bir_lowering=False, debug=False,
                   num_devices=N_CORES)

    qT = nc.dram_tensor("qT", [D, SQ], F32R, kind="ExternalInput")
    kT = nc.dram_tensor("kT", [D, S], F32R, kind="ExternalInput")
    wqT = nc.dram_tensor("wqT", [D, D], F32R, kind="ExternalInput")
    wkT = nc.dram_tensor("wkT", [D, D], F32R, kind="ExternalInput")
    wvT = nc.dram_tensor("wvT", [D, D], F32R, kind="ExternalInput")
    out = nc.dram_tensor("out", [SQ, D], F32, kind="ExternalOutput")

    from concourse.masks import make_identity

    with tile.TileContext(nc) as tc:
        with ExitStack() as ctx:
            psum = ctx.enter_context(tc.tile_pool(name="psum", bufs=6, space="PSUM"))
            psl = ctx.enter_context(tc.tile_pool(name="psl", bufs=1, space="PSUM"))
            dram = ctx.enter_context(tc.tile_pool(name="dram", bufs=1, space="DRAM"))
            consts = ctx.enter_context(tc.tile_pool(name="consts", bufs=1))
            maxp = ctx.enter_context(tc.tile_pool(name="maxp", bufs=1))
            qtp = ctx.enter_context(tc.tile_pool(name="qtp", bufs=NE))

            id8 = consts.tile([8, 8], F32)
            make_identity(nc, id8[:])

            macc = maxp.tile([P, SQ], F32)
            stbp = ctx.enter_context(tc.tile_pool(name="stbp", bufs=3))
            tmp = maxp.tile([64, SQ], F32)
            m_bc = maxp.tile([P, SQ], F32)

            st_spill = [dram.tile([P, SQ], F32, tag="stsp", name=f"stsp{i}")
                        for i in range(NK)]
            v_spill = [dram.tile([P, D], BF16, tag="vsp", name=f"vsp{i}")
                       for i in range(NK)]

            # ---- P1: QT[e,q] projection; stays resident in SBUF -------
            # kin (the f32r key tiles) opens early so its 8MB load runs
            # on the sync queue during the P1 matmuls.
            kin_ctx = ExitStack()
            kin = kin_ctx.enter_context(tc.tile_pool(name="kin", bufs=ND))
            kts = [kin.tile([P, S], F32R, tag="kin", name=f"kin{i}")
                   for i in range(ND)]
            QTr = [qtp.tile([P, SQ], F32R, tag="qtr", name=f"qtr{i}")
                   for i in range(NE)]
            with tc.tile_pool(name="qin", bufs=ND) as qin, \
                 tc.tile_pool(name="wq", bufs=ND) as wq:
                qts = [qin.tile([P, SQ], F32R, tag="qin", name=f"qin{i}")
                       for i in range(ND)]
                wqs = [wq.tile([P, D], F32R, tag="wq", name=f"wq{i}")
                       for i in range(ND)]
                for d in range(ND):
                    nc.gpsimd.dma_start(wqs[d][:], wqT.ap()[ts(d, P), :])
                for d in range(ND):
                    eng = nc.sync if d < 6 else nc.scalar
                    eng.dma_start(qts[d][:], qT.ap()[ts(d, P), :])
                for d in range(ND):
                    nc.sync.dma_start(kts[d][:], kT.ap()[ts(d, P), :])
                for e in range(NE):
                    for qh in range(SQ // 512):
                        ps = psum.tile([P, 512], F32, tag="mm", name=f"ps_q{e}_{qh}")
                        for d in range(ND):
                            nc.tensor.matmul(ps[:], wqs[d][:, ts(e, P)],
                                             qts[d][:, ts(qh, 512)],
                                             start=(d == 0), stop=(d == ND - 1))
                        nc.vector.tensor_copy(QTr[e][:, ts(qh, 512)], ps[:])

            st_res = {}
            # ---- P2+P3 fused: KT chunks feed score matmuls directly ----
            # For each kc (512 keys): project KT[e, kc] for all e, then
            # immediately run the 4 k-tiles of scores using those chunks
            # as lhsT straight from SBUF. No KT spill. V projection (also
            # f32r, reusing the resident kT tiles) follows the score loop
            # in the PE stream so the PE stays busy during the softmax
            # reductions; V spills to DRAM and reloads into freed SBUF.
            with tc.tile_pool(name="wk", bufs=ND) as wk, \
                 tc.tile_pool(name="wvr", bufs=ND) as wvr, \
                 tc.tile_pool(name="ktc", bufs=NE) as ktc, \
                 tc.tile_pool(name="vb", bufs=3) as vb:
                wks = [wk.tile([P, D], F32R, tag="wk", name=f"wk{i}")
                       for i in range(ND)]
                wvs = [wvr.tile([P, D], F32R, tag="wvr", name=f"wvr{i}")
                       for i in range(ND)]
                for d in range(ND):
                    nc.gpsimd.dma_start(wks[d][:], wkT.ap()[ts(d, P), :])
                for d in range(ND):
                    nc.gpsimd.dma_start(wvs[d][:], wvT.ap()[ts(d, P), :])

                for kc in range(S // 512):
                    ktcs = []
                    for e in range(NE):
                        ps = psum.tile([P, 512], F32, tag="mm",
                                       name=f"ps_k{e}_{kc}")
                        for d in range(ND):
                            nc.tensor.matmul(ps[:], wks[d][:, ts(e, P)],
                                             kts[d][:, ts(kc, 512)],
                                             start=(d == 0), stop=(d == ND - 1))
                        kt_c = ktc.tile([P, 512], F32R, tag="ktc",
                                        name=f"ktc{e}_{kc}")
                        nc.vector.tensor_copy(kt_c[:], ps[:])
                        ktcs.append(kt_c)
                    for kk in range(4):          # 4 k-tiles inside this kc
                        k = kc * 4 + kk
                        st_k = stbp.tile([P, SQ], F32, tag="stb", name=f"stb{k}")
                        for qh in range(SQ // 512):
                            ps = psum.tile([P, 512], F32, tag="mm",
                                           name=f"ps_s{k}_{qh}")
                            for e in range(NE):
                                nc.tensor.matmul(ps[:], ktcs[e][:, ts(kk, P)],
                                                 QTr[e][:, ts(qh, 512)],
                                                 start=(e == 0),
                                                 stop=(e == NE - 1))
                            nc.vector.tensor_copy(st_k[:, ts(qh, 512)], ps[:])
                        if k == 0:
                            nc.vector.tensor_copy(macc[:], st_k[:])
                        else:
                            nc.vector.tensor_max(macc[:], macc[:], st_k[:])
                        if k < NK - 3:
                            nc.sync.dma_start(st_spill[k][:], st_k[:])
                        else:
                            st_res[k] = st_k

                # V projection: no softmax deps -> fills PE during reduce
                for k in range(NK):
                    for eh in range(D // 512):
                        ps = psum.tile([P, 512], F32, tag="mm",
                                       name=f"ps_v{k}_{eh}")
                        for d in range(ND):
                            nc.tensor.matmul(ps[:], kts[d][:, ts(k, P)],
                                             wvs[d][:, ts(eh, 512)],
                                             start=(d == 0), stop=(d == ND - 1))
                        v_c = vb.tile([P, 512], BF16, tag="vb",
                                      name=f"vb{k}_{eh}")
                        nc.scalar.copy(v_c[:], ps[:])
                        nc.sync.dma_start(v_spill[k][:, ts(eh, 512)], v_c[:])

            kin_ctx.close()

            # ---- V reload into SBUF freed by the projection inputs ----
            vp = ctx.enter_context(tc.tile_pool(name="vp", bufs=NK))
            V = [vp.tile([P, D], BF16, tag="v", name=f"v{i}") for i in range(NK)]
            for k in range(NK):
                nc.sync.dma_start(V[k][:], v_spill[k][:])

            # ---- P3b: partition halving max -> row max broadcast ------
            # broadcast via DRAM bounce + zero-partition-stride reload:
            # no PE involvement, so it completes under the V matmuls.
            w = 64
            while w >= 1:
                nc.sync.dma_start(tmp[0:w, :], macc[w:2 * w, :])
                nc.vector.tensor_max(macc[0:w, :], macc[0:w, :], tmp[0:w, :])
                w //= 2
            m_dram = dram.tile([1, SQ], F32)
            nc.sync.dma_start(m_dram[:], macc[0:1, :])
            nc.sync.dma_start(m_bc[:], m_dram[0:1, :].to_broadcast([P, SQ]))

            # ---- P4a: E = exp(ST - m) in bf16; accumulate l inline ----
            # ST reloads land in the freed QTr slots (tag reuse) so they
            # can start as soon as the score matmuls finish; the subtract
            # runs in place to avoid extra tiles.
            ep = ctx.enter_context(tc.tile_pool(name="ep", bufs=NK))
            mx2p = ctx.enter_context(tc.tile_pool(name="mx2p", bufs=1))
            lacc = mx2p.tile([P, SQ], F32)
            E = [ep.tile([P, SQ], BF16, tag="e", name=f"e{i}") for i in range(NK)]
            # Half-tile (512-col) sub/exp pipeline: the first PV groups
            # (qc 0..3) only read E columns 0..511, so producing all the
            # first halves before any second half lets PV start after
            # half the serial exp chain.
            korder = [NK - 3, NK - 2, NK - 1] + list(range(NK - 3))
            st_tiles = {}
            for k in korder:
                if k in st_res:
                    st_tiles[k] = st_res[k]
                else:
                    st_k = qtp.tile([P, SQ], F32, tag="qtr",
                                    name=f"stin{k}")
                    nc.gpsimd.dma_start(st_k[:], st_spill[k][:])
                    st_tiles[k] = st_k
            for qh in range(SQ // 512):
                sl = ts(qh, 512)
                for i, k in enumerate(korder):
                    st_k = st_tiles[k]
                    nc.vector.tensor_sub(st_k[:, sl], st_k[:, sl],
                                         m_bc[:, sl])
                    nc.scalar.activation(E[k][:, sl], st_k[:, sl],
                                         mybir.ActivationFunctionType.Exp)
                    if i == 1:
                        nc.vector.tensor_add(lacc[:, sl],
                                             E[korder[0]][:, sl],
                                             E[korder[1]][:, sl])
                    elif i > 1:
                        nc.vector.tensor_add(lacc[:, sl], lacc[:, sl],
                                             E[k][:, sl])

            # ---- P4c: l[q] row sums on DVE; reciprocal to [q,1] layout -
            # lacc = sum over k-tiles of E (fp32 accumulate from bf16),
            # partition-halving sum -> l_row[1, SQ], reciprocal, reshape
            # to [8, 128] via 8 row DMAs, PE-transpose -> recip_t[128, 8];
            # column qc is then the per-partition 1/l for q-chunk qc.
            # ---- P4d: O' = E.T @ V (drains independent of 1/l), then
            # PE-transpose 1/l AFTER the PV matmuls and scale on store ---
            outp = ctx.enter_context(tc.tile_pool(name="outp", bufs=2 * NQC))
            ones_c = consts.tile([P, 1], F32)
            nc.gpsimd.memset(ones_c[:], 1.0)

            # The l row-sum matmuls + PE transpose are threaded into the
            # middle of the PV loop so the whole 1/l path completes while
            # the PE is still doing PV matmuls; drained output chunks are
            # scaled and stored as soon as recip_t exists.
            groups = [(qc, eh) for qc in range(NQC) for eh in range(D // 512)]
            l_row = mx2p.tile([1, SQ], F32)
            r_dram = dram.tile([1, SQ], F32)
            r8 = mx2p.tile([8, P], F32)
            pt8 = psl.tile([P, 8], F32, tag="pt8")
            recip_t = mx2p.tile([P, 8], F32)
            pending = []

            def emit_store(qc, eh, ot, i):
                nc.vector.tensor_scalar_mul(ot[:], ot[:], recip_t[:, qc:qc + 1])
                eng = nc.sync if i % 2 == 0 else nc.scalar
                eng.dma_start(out.ap()[ts(qc, P), ts(eh, 512)], ot[:])

            for g, (qc, eh) in enumerate(groups):
                ps = psum.tile([P, 512], F32, tag="mm", name=f"ps_o{qc}_{eh}")
                for k in range(NK):
                    nc.tensor.matmul(ps[:], E[k][:, ts(qc, P)],
                                     V[k][:, ts(eh, 512)],
                                     start=(k == 0), stop=(k == NK - 1))
                ot = outp.tile([P, 512], F32, tag="ot", name=f"ot{qc}_{eh}")
                nc.vector.tensor_copy(ot[:], ps[:])
                if g < 11:
                    pending.append((qc, eh, ot))
                else:
                    emit_store(qc, eh, ot, g)
                if g == 8:
                    for qh in range(SQ // 512):
                        pl = psl.tile([1, 512], F32, tag="pl", name=f"pl{qh}")
                        nc.tensor.matmul(pl[:], ones_c[:], lacc[:, ts(qh, 512)],
                                         start=True, stop=True)
                        nc.vector.tensor_copy(l_row[0:1, ts(qh, 512)], pl[:])
                    nc.sync.dma_start(r_dram[:], l_row[:])
                    nc.sync.dma_start(
                        r8[:], r_dram[0, :].rearrange("(a b) -> a b", a=8))
                elif g == 10:
                    nc.tensor.transpose(pt8[:], r8[:], id8[:])
                    nc.vector.reciprocal(recip_t[:], pt8[:])
                    for i, (pqc, peh, pot) in enumerate(pending):
                        emit_store(pqc, peh, pot, i)

    nc.compile()
    _NC_CACHE["nc"] = nc
    return nc


def kernel(query, key, Wq, Wk, Wv):
    query = np.asarray(query, dtype=np.float32)
    key = np.asarray(key, dtype=np.float32)
    wqT = np.ascontiguousarray(np.asarray(Wq, dtype=np.float32).T)
    wkT = np.ascontiguousarray(np.asarray(Wk, dtype=np.float32).T)
    wvT_np = np.ascontiguousarray(np.asarray(Wv, dtype=np.float32).T)

    in_maps = []
    for c in range(N_CORES):
        b, h = c // 2, c % 2
        qTn = np.ascontiguousarray(query[b, h * SQ:(h + 1) * SQ, :].T)
        kTn = np.ascontiguousarray(key[b].T)
        in_maps.append({
            "qT": qTn, "kT": kTn, "wqT": wqT, "wkT": wkT, "wvT": wvT_np,
        })

    nc = _build()
    res = run_bass_kernel_spmd(nc, in_maps, core_ids=list(range(N_CORES)))
    outv = np.empty((B, S, D), dtype=np.float32)
    for c in range(N_CORES):
        b, h = c // 2, c % 2
        outv[b, h * SQ:(h + 1) * SQ, :] = res.results[c]["out"]
    return outv



# revision 4
# speedup vs baseline: 1.0705x; 1.0705x over previous
"""Trainium2 Bass kernel: attention layer (B=4, S=2048, D=1024), 8 NeuronCores.

Sharding: data-parallel over (batch, query-half) -> 8 shards. Each core
computes one batch's half of the queries against that batch's full keys.

Algorithm (reassociated to cut PE work ~22% vs the direct form):
  scores = (q Wq^T)(key Wk^T)^T = ((q Wq^T) Wk) key^T
    -> transform the 1024 queries twice (2x 128 matmuls) instead of
       projecting the 2048 keys (256 matmuls); the score matmul then
       contracts the RAW key (kT input, f32r) with AT.
  out = P (key Wv^T) = (P key) Wv^T
    -> contract P with the raw key first (U^T = key^T E, 256 matmuls),
       then one 1024x1024 transform (128 matmuls) instead of projecting
       V for all 2048 keys (256) + PV (256). V is never materialized.
This removes every matmul that was redundant between the two cores of a
batch pair, with zero communication. ~898 matmuls/core vs 1155.

Per-core dataflow:
  P1  QT[e,q]  = Wq qT        (f32r; d-outer sweeps of 4 e-groups so the
                               first matmul only needs one wq/qt tile --
                               inputs round-robin across 3 DMA queues)
  P1b AT[d',q] = Wk^T QT      (f32r; lhsT = raw Wk tiles)
  P2  ST[k,q]  = kT^T AT      (f32r; k on partitions; row-max tracked by
                               DVE max chain; first 8 k-tiles spilled to
                               DRAM, 8 stay resident)
  P3  softmax max along k (partition dim): 7-step DMA-shift halving
      -> m[1,q] -> DRAM bounce -> broadcast reload [128,q]
      (spilled ST reloads prefetched under this, no dependency on m)
  P4  E = exp(ST - m) in bf16, qh-half-major, resident k-tiles first so
      the spilled-reload DMAs hide under the resident exps
  P5  UT[d,q]  = key^T E      (bf16; lhsT = raw bf16 key tiles; 4
                               concurrent PSUM groups, k-outer, so PE
                               consumption paces the exp chain)
  P6  O[q,e]   = UT^T Wv^T    (bf16; the l ones-matmuls, PE-transpose of
                               1/l, and scaled stores threaded in)

SBUF is managed as flat always-open pools of uniform 4KB/partition
slots with tag-rotation reuse (victim's last read always precedes the
new tile's first write):
  A(64KB,16): kT halves        -> kraw bf16 pairs (8) -> E pairs (8)
  B(32KB, 8): qT tiles         -> Wk tiles  -> m_bc,macc,lacc,tmp -> O
  C(32KB, 8): Wq tiles         -> AT        -> ST reloads
  D(32KB, 8): QT               -> ST residents (k=8..15)
  W(32KB, 8): wv bf16 pairs (4) + UT bf16 pairs (4)
Total ~197KB of ~203 usable. float32r keeps ~1.5e-4 relative precision
on the unscaled (logit std ~34) softmax path; bf16 is fine for E and
the U/O contractions.
"""

import numpy as np
import ml_dtypes
from contextlib import ExitStack

import concourse.bass as bass
import concourse.tile as tile
from concourse import bacc, mybir
from concourse.bass import ts
from concourse.bass_utils import run_bass_kernel_spmd

B, S, D = 4, 2048, 1024
N_CORES = 8
SQ = S // 2            # 1024 query rows per core
P = 128                # partitions
NE = D // P            # 8 e-tiles
ND = D // P            # 8 d-tiles
NK = S // P            # 16 k-tiles
NQC = SQ // P          # 8 q-chunks
NSPILL = 8             # ST k-tiles spilled to DRAM (rest stay resident)
F32R = mybir.dt.float32r
F32 = mybir.dt.float32
BF16 = mybir.dt.bfloat16

# E/UT production+consumption order: resident ST tiles (k=8..15) first,
# so the spilled-tile reload DMAs run under the resident exps.
KORDER = list(range(NSPILL, NK)) + list(range(NSPILL))

_NC_CACHE = {}


def _build():
    if "nc" in _NC_CACHE:
        return _NC_CACHE["nc"]
    nc = bacc.Bacc("TRN2", target_bir_lowering=False, debug=False,
                   num_devices=N_CORES)

    qT = nc.dram_tensor("qT", [D, SQ], F32R, kind="ExternalInput")
    kT = nc.dram_tensor("kT", [D, S], F32R, kind="ExternalInput")
    kraw = nc.dram_tensor("kraw", [S, D], BF16, kind="ExternalInput")
    wqT = nc.dram_tensor("wqT", [D, D], F32R, kind="ExternalInput")
    wk = nc.dram_tensor("wk", [D, D], F32R, kind="ExternalInput")
    wvT = nc.dram_tensor("wvT", [D, D], BF16, kind="ExternalInput")
    out = nc.dram_tensor("out", [SQ, D], F32, kind="ExternalOutput")

    from concourse.masks import make_identity

    with tile.TileContext(nc) as tc:
        with ExitStack() as ctx:
            psum = ctx.enter_context(tc.tile_pool(name="psum", bufs=6, space="PSUM"))
            psl = ctx.enter_context(tc.tile_pool(name="psl", bufs=1, space="PSUM"))
            dram = ctx.enter_context(tc.tile_pool(name="dram", bufs=1, space="DRAM"))
            consts = ctx.enter_context(tc.tile_pool(name="consts", bufs=1))
            tiny = ctx.enter_context(tc.tile_pool(name="tiny", bufs=1))
            pA = ctx.enter_context(tc.tile_pool(name="pA", bufs=16))
            pB = ctx.enter_context(tc.tile_pool(name="pB", bufs=8))
            pC = ctx.enter_context(tc.tile_pool(name="pC", bufs=8))
            pD = ctx.enter_context(tc.tile_pool(name="pD", bufs=8))
            pW = ctx.enter_context(tc.tile_pool(name="pW", bufs=8))

            id8 = consts.tile([8, 8], F32)
            make_identity(nc, id8[:])
            ones_c = consts.tile([P, 1], F32)
            nc.gpsimd.memset(ones_c[:], 1.0)

            st_spill = [dram.tile([P, SQ], F32, tag="stsp", name=f"stsp{i}")
                        for i in range(NSPILL)]

            dmae = [nc.sync, nc.scalar, nc.gpsimd]

            # ---- tile allocation in rotation order -----------------
            qts = [pB.tile([P, SQ], F32R, tag="b", name=f"qin{i}")
                   for i in range(ND)]
            wqs = [pC.tile([P, D], F32R, tag="c", name=f"wq{i}")
                   for i in range(ND)]
            ktsA = [pA.tile([P, SQ], F32R, tag="a", name=f"kta{i}")
                    for i in range(ND)]
            ktsB = [pA.tile([P, SQ], F32R, tag="a", name=f"ktb{i}")
                    for i in range(ND)]
            QT = [pD.tile([P, SQ], F32R, tag="d", name=f"qt{i}")
                  for i in range(NE)]
            wks = [pB.tile([P, D], F32R, tag="b", name=f"wk{i}")
                   for i in range(ND)]

            # input loads, consumption-ordered, round-robin 3 queues.
            # wk writes into B (qts victims): the queues head-of-line
            # block on P1's last qts read, so wk goes LAST.
            for d in range(ND):
                dmae[(2 * d) % 3].dma_start(wqs[d][:], wqT.ap()[ts(d, P), :])
                dmae[(2 * d + 1) % 3].dma_start(qts[d][:], qT.ap()[ts(d, P), :])
            for d in range(ND):
                dmae[(2 * d) % 3].dma_start(ktsA[d][:], kT.ap()[ts(d, P), 0:SQ])
                dmae[(2 * d + 1) % 3].dma_start(ktsB[d][:], kT.ap()[ts(d, P), SQ:S])
            for d in range(ND):
                dmae[d % 3].dma_start(wks[d][:], wk.ap()[ts(d, P), :])

            # ---- P1: QT[e,q] = Wq qT -------------------------------
            # d-outer over 4-e groups: the first matmul needs only
            # wqs[0]+qts[0]; each arriving d-tile feeds 4 matmuls, so
            # the PE saturates at the 3-queue DMA arrival rate.
            for qh in range(2):
                for eg in range(2):
                    pss = [psum.tile([P, 512], F32, tag="mm",
                                     name=f"ps_q{qh}_{eg}_{i}")
                           for i in range(4)]
                    for d in range(ND):
                        for i in range(4):
                            nc.tensor.matmul(pss[i][:],
                                             wqs[d][:, ts(eg * 4 + i, P)],
                                             qts[d][:, ts(qh, 512)],
                                             start=(d == 0),
                                             stop=(d == ND - 1))
                    for i in range(4):
                        nc.vector.tensor_copy(
                            QT[eg * 4 + i][:, ts(qh, 512)], pss[i][:])

            # ---- P1b: AT[d',q] = Wk^T QT ---------------------------
            AT = [pC.tile([P, SQ], F32R, tag="c", name=f"at{i}")
                  for i in range(ND)]
            for qh in range(2):
                for dg in range(2):
                    pss = [psum.tile([P, 512], F32, tag="mm",
                                     name=f"ps_a{qh}_{dg}_{i}")
                           for i in range(4)]
                    for e in range(NE):
                        for i in range(4):
                            nc.tensor.matmul(pss[i][:],
                                             wks[e][:, ts(dg * 4 + i, P)],
                                             QT[e][:, ts(qh, 512)],
                                             start=(e == 0),
                                             stop=(e == NE - 1))
                    for i in range(4):
                        nc.vector.tensor_copy(
                            AT[dg * 4 + i][:, ts(qh, 512)], pss[i][:])

            # softmax scratch lands in B (wk victims, dead after P1b)
            m_bc = pB.tile([P, SQ], F32, tag="b", name="m_bc")
            macc = pB.tile([P, SQ], F32, tag="b", name="macc")
            lacc = pB.tile([P, SQ], F32, tag="b", name="lacc")
            tmp = pB.tile([P, SQ], F32, tag="b", name="tmp")

            # ---- P2: ST[k,q] = kT^T AT; DVE row-max on the fly -----
            # k=0..7 spill to DRAM (slot reused 8 tiles later by the
            # rotation); k=8..15 stay resident in D (QT victims).
            st_tiles = {}
            for k in range(NK):
                st_k = pD.tile([P, SQ], F32, tag="d", name=f"stb{k}")
                kts = ktsA if k < 8 else ktsB
                kk = k % 8
                for qh in range(2):
                    ps = psum.tile([P, 512], F32, tag="mm",
                                   name=f"ps_s{k}_{qh}")
                    for dp in range(ND):
                        nc.tensor.matmul(ps[:], kts[dp][:, ts(kk, P)],
                                         AT[dp][:, ts(qh, 512)],
                                         start=(dp == 0),
                                         stop=(dp == ND - 1))
                    nc.vector.tensor_copy(st_k[:, ts(qh, 512)], ps[:])
                if k == 0:
                    nc.vector.tensor_copy(macc[:], st_k[:])
                else:
                    nc.vector.tensor_max(macc[:], macc[:], st_k[:])
                if k < NSPILL:
                    nc.sync.dma_start(st_spill[k][:], st_k[:])
                else:
                    st_tiles[k] = st_k

            # ---- P3: partition-halving max -> broadcast ------------
            # Reloads (AT victims; no dependency on m) prefetch on the
            # gpsimd queue under the reduction; kraw/wv loads follow on
            # sync behind the tiny halving/bounce DMAs (needed only
            # from P5/P6, consumed in KORDER so k=8.. goes first).
            for k in range(NSPILL):
                st_k = pC.tile([P, SQ], F32, tag="c", name=f"rl{k}")
                nc.gpsimd.dma_start(st_k[:], st_spill[k][:])
                st_tiles[k] = st_k
            w = 64
            while w >= 1:
                nc.sync.dma_start(tmp[0:w, :], macc[w:2 * w, :])
                nc.vector.tensor_max(macc[0:w, :], macc[0:w, :], tmp[0:w, :])
                w //= 2
            m_dram = dram.tile([1, SQ], F32)
            nc.sync.dma_start(m_dram[:], macc[0:1, :])
            nc.sync.dma_start(m_bc[:], m_dram[0:1, :].to_broadcast([P, SQ]))

            krs2 = [pA.tile([P, 2 * SQ], BF16, tag="a", name=f"kr{j}")
                    for j in range(NK // 2)]
            for j in range(NK // 2):
                for h in range(2):
                    k = KORDER[2 * j + h]
                    nc.sync.dma_start(krs2[j][:, ts(h, SQ)],
                                      kraw.ap()[ts(k, P), :])
            wv2 = [pW.tile([P, 2 * SQ], BF16, tag="w", name=f"wv{j}")
                   for j in range(ND // 2)]
            for j in range(ND // 2):
                for h in range(2):
                    nc.sync.dma_start(wv2[j][:, ts(h, SQ)],
                                      wvT.ap()[ts(2 * j + h, P), :])

            def kr_ap(i, dp):       # lhsT slice for KORDER[i], d-tile dp
                return krs2[i // 2][:, (i % 2) * SQ + dp * P:
                                    (i % 2) * SQ + (dp + 1) * P]

            # ---- P4+P5 interleaved: E = exp(ST - m) bf16, l on DVE;
            # UT[d,q] = key^T E, 4 concurrent PSUM groups, k-outer ----
            E2 = [pA.tile([P, 2 * SQ], BF16, tag="a", name=f"e{j}")
                  for j in range(NK // 2)]

            def e_ap(i, qh):        # [P,512] E view for KORDER[i]
                return E2[i // 2][:, (i % 2) * SQ + qh * 512:
                                  (i % 2) * SQ + qh * 512 + 512]

            ut2 = [pW.tile([P, 2 * SQ], BF16, tag="w", name=f"ut{j}")
                   for j in range(ND // 2)]

            def ut_ap(dp, c0, w_):  # [P,w_] UT view for d-tile dp
                return ut2[dp // 2][:, (dp % 2) * SQ + c0:
                                    (dp % 2) * SQ + c0 + w_]

            for qh in range(2):
                sl = ts(qh, 512)
                for i, k in enumerate(KORDER):
                    st_k = st_tiles[k]
                    nc.vector.tensor_sub(st_k[:, sl], st_k[:, sl],
                                         m_bc[:, sl])
                    nc.scalar.activation(e_ap(i, qh), st_k[:, sl],
                                         mybir.ActivationFunctionType.Exp)
                    if i == 1:
                        nc.vector.tensor_add(lacc[:, sl], e_ap(0, qh),
                                             e_ap(1, qh))
                    elif i > 1:
                        nc.vector.tensor_add(lacc[:, sl], lacc[:, sl],
                                             e_ap(i, qh))
                for dg in range(2):
                    pss = [psum.tile([P, 512], F32, tag="mm",
                                     name=f"ps_u{qh}_{dg}_{i}")
                           for i in range(4)]
                    for i in range(NK):
                        for t in range(4):
                            nc.tensor.matmul(pss[t][:],
                                             kr_ap(i, dg * 4 + t),
                                             e_ap(i, qh),
                                             start=(i == 0),
                                             stop=(i == NK - 1))
                    for t in range(4):
                        nc.vector.tensor_copy(
                            ut_ap(dg * 4 + t, qh * 512, 512), pss[t][:])

            # ---- P6: O[q,e] = UT^T Wv^T; 1/l path threaded in ------
            groups = [(qc, eh) for qc in range(NQC) for eh in range(D // 512)]
            l_row = tiny.tile([1, SQ], F32)
            r_dram = dram.tile([1, SQ], F32)
            r8 = tiny.tile([8, P], F32)
            pt8 = psl.tile([P, 8], F32, tag="pt8")
            recip_t = tiny.tile([P, 8], F32)
            pending = []

            def emit_store(qc, eh, ot, i):
                nc.vector.tensor_scalar_mul(ot[:], ot[:], recip_t[:, qc:qc + 1])
                eng = nc.sync if i % 2 == 0 else nc.scalar
                eng.dma_start(out.ap()[ts(qc, P), ts(eh, 512)], ot[:])

            def wv_ap(dp, eh):      # [P,512] Wv^T view for d-tile dp
                return wv2[dp // 2][:, (dp % 2) * SQ + eh * 512:
                                    (dp % 2) * SQ + eh * 512 + 512]

            for g, (qc, eh) in enumerate(groups):
                ps = psum.tile([P, 512], F32, tag="mm", name=f"ps_o{qc}_{eh}")
                for dp in range(ND):
                    nc.tensor.matmul(ps[:], ut_ap(dp, qc * P, P),
                                     wv_ap(dp, eh),
                                     start=(dp == 0), stop=(dp == ND - 1))
                ot = pB.tile([P, 512], F32, tag="b", name=f"ot{qc}_{eh}")
                nc.vector.tensor_copy(ot[:], ps[:])
                if g < 5:
                    pending.append((qc, eh, ot))
                else:
                    emit_store(qc, eh, ot, g)
                if g == 2:
                    for qh in range(2):
                        pl = psl.tile([1, 512], F32, tag="pl", name=f"pl{qh}")
                        nc.tensor.matmul(pl[:], ones_c[:], lacc[:, ts(qh, 512)],
                                         start=True, stop=True)
                        nc.vector.tensor_copy(l_row[0:1, ts(qh, 512)], pl[:])
                    nc.sync.dma_start(r_dram[:], l_row[:])
                    nc.sync.dma_start(
                        r8[:], r_dram[0, :].rearrange("(a b) -> a b", a=8))
                elif g == 4:
                    nc.tensor.transpose(pt8[:], r8[:], id8[:])
                    nc.vector.reciprocal(recip_t[:], pt8[:])
                    for i, (pqc, peh, pot) in enumerate(pending):
                        emit_store(pqc, peh, pot, i)

    nc.compile()
    _NC_CACHE["nc"] = nc
    return nc


def make_in_maps(query, key, Wq, Wk, Wv):
    query = np.asarray(query, dtype=np.float32)
    key = np.asarray(key, dtype=np.float32)
    wqT = np.ascontiguousarray(np.asarray(Wq, dtype=np.float32).T)
    wk_np = np.ascontiguousarray(np.asarray(Wk, dtype=np.float32))
    wvT_np = np.ascontiguousarray(
        np.asarray(Wv, dtype=np.float32).T.astype(ml_dtypes.bfloat16))

    in_maps = []
    for c in range(N_CORES):
        b, h = c // 2, c % 2
        qTn = np.ascontiguousarray(query[b, h * SQ:(h + 1) * SQ, :].T)
        kTn = np.ascontiguousarray(key[b].T)
        krn = np.ascontiguousarray(key[b].astype(ml_dtypes.bfloat16))
        in_maps.append({
            "qT": qTn, "kT": kTn, "kraw": krn,
            "wqT": wqT, "wk": wk_np, "wvT": wvT_np,
        })
    return in_maps


def kernel(query, key, Wq, Wk, Wv):
    in_maps = make_in_maps(query, key, Wq, Wk, Wv)
    nc = _build()
    res = run_bass_kernel_spmd(nc, in_maps, core_ids=list(range(N_CORES)))
    outv = np.empty((B, S, D), dtype=np.float32)
    for c in range(N_CORES):
        b, h = c // 2, c % 2
        outv[b, h * SQ:(h + 1) * SQ, :] = res.results[c]["out"]
    return outv


# revision 6
# speedup vs baseline: 1.2829x; 1.1984x over previous
"""Trainium2 Bass kernel: attention layer (B=4, S=2048, D=1024), 8 NeuronCores.

Sharding: data-parallel over (batch, query-half) -> 8 shards. Each core
computes one batch's half of the queries against that batch's full keys.

Algorithm (reassociated to cut PE work ~22% vs the direct form):
  scores = (q Wq^T)(key Wk^T)^T = q (Wq^T Wk) key^T
    -> MT = Wq^T Wk (128 matmuls, needs only the two weight matrices so
       the PE starts ~2 tile-loads into the kernel), AT = MT qT (128),
       then scores contract the RAW key (kT input, f32r) with AT (256)
       instead of Q-proj(128) + K-proj(256) + scores(256).
  out = P (key Wv^T) = (P key) Wv^T
    -> contract P with the raw key first (U^T = key^T E, 256 matmuls),
       then one 1024x1024 transform (128 matmuls) instead of projecting
       V for all 2048 keys (256) + PV (256). V is never materialized.
This removes every matmul that was redundant between the two cores of a
batch pair, with zero communication. ~898 matmuls/core vs 1155.

Per-core dataflow:
  P0  MT[d,d'] = Wq^T Wk      (f32r; e-inner sweeps of 4 d-groups pace
                               the PE at the wq/wk DMA arrival rate)
  P1  AT[d',q] = MT qT        (f32r; qT arrives during P0)
  P2  ST[k,q]  = kT^T AT      (f32r; k on partitions; row-max tracked by
                               DVE max chain; first 8 k-tiles spill to
                               DRAM, 8 stay resident; after the k=7
                               chain the kraw/wv loads and ST reloads
                               are issued so every DMA lands before the
                               softmax window)
  P3  m_bc = gpsimd.partition_all_reduce(max) over macc -- replaces the
      7-step DMA-shift halving + DRAM bounce broadcast of the baseline
      (whose serial semaphore+DMA latency cost ~40us of PE idle)
  P4  E = exp(ST - m) in bf16, qh-half-major; l accumulated on DVE
  P5  UT[d,q]  = key^T E      (bf16; lhsT = raw bf16 key tiles; 4
                               concurrent PSUM groups, k-outer, so PE
                               consumption paces the exp chain)
  P6  O[q,e]   = UT^T Wv^T    (bf16; the l ones-matmuls, PE-transpose of
                               1/l via DRAM bounce, and scaled stores
                               threaded into the O loop)

SBUF is managed as flat always-open pools of uniform 4KB/partition
slots with tag-rotation reuse (victim's last read always precedes the
new tile's first write):
  A(64KB,16): kT halves     -> kraw bf16 pairs (8) -> E pairs (8)
  B(32KB, 8): Wq tiles      -> m_bc,macc,lacc,(spare) -> O out-stage
  C(32KB, 8): Wk tiles      -> AT -> wv bf16 pairs (4) + UT pairs (4)
  D(32KB, 8): MT            -> ST residents (k=8..15)
  Q(32KB, 8): qT tiles      -> ST reloads (k=0..7)
Total ~198KB of ~203 usable. float32r keeps ~1.5e-4 relative precision
on the unscaled (logit std ~34) softmax path; bf16 is fine for E and
the U/O contractions.
"""

import numpy as np
import ml_dtypes
from contextlib import ExitStack

import concourse.bass as bass
import concourse.tile as tile
from concourse import bacc, bass_isa, mybir
from concourse.bass import ts
from concourse.bass_utils import run_bass_kernel_spmd

B, S, D = 4, 2048, 1024
N_CORES = 8
SQ = S // 2            # 1024 query rows per core
P = 128                # partitions
NE = D // P            # 8 e-tiles
ND = D // P            # 8 d-tiles
NK = S // P            # 16 k-tiles
NQC = SQ // P          # 8 q-chunks
NSPILL = 8             # ST k-tiles spilled to DRAM (rest stay resident)
F32R = mybir.dt.float32r
F32 = mybir.dt.float32
BF16 = mybir.dt.bfloat16

# E/UT production+consumption order: resident ST tiles (k=8..15) first.
KORDER = list(range(NSPILL, NK)) + list(range(NSPILL))

_NC_CACHE = {}


def _build():
    if "nc" in _NC_CACHE:
        return _NC_CACHE["nc"]
    nc = bacc.Bacc("TRN2", target_bir_lowering=False, debug=False,
                   num_devices=N_CORES)

    qT = nc.dram_tensor("qT", [D, SQ], F32R, kind="ExternalInput")
    kT = nc.dram_tensor("kT", [D, S], F32R, kind="ExternalInput")
    kraw = nc.dram_tensor("kraw", [S, D], BF16, kind="ExternalInput")
    wq = nc.dram_tensor("wq", [D, D], F32R, kind="ExternalInput")
    wk = nc.dram_tensor("wk", [D, D], F32R, kind="ExternalInput")
    wvT = nc.dram_tensor("wvT", [D, D], BF16, kind="ExternalInput")
    out = nc.dram_tensor("out", [SQ, D], F32, kind="ExternalOutput")

    from concourse.masks import make_identity

    with tile.TileContext(nc) as tc:
        with ExitStack() as ctx:
            psum = ctx.enter_context(tc.tile_pool(name="psum", bufs=6, space="PSUM"))
            psl = ctx.enter_context(tc.tile_pool(name="psl", bufs=1, space="PSUM"))
            dram = ctx.enter_context(tc.tile_pool(name="dram", bufs=1, space="DRAM"))
            consts = ctx.enter_context(tc.tile_pool(name="consts", bufs=1))
            tiny = ctx.enter_context(tc.tile_pool(name="tiny", bufs=1))
            pA = ctx.enter_context(tc.tile_pool(name="pA", bufs=16))
            pB = ctx.enter_context(tc.tile_pool(name="pB", bufs=8))
            pC = ctx.enter_context(tc.tile_pool(name="pC", bufs=8))
            pD = ctx.enter_context(tc.tile_pool(name="pD", bufs=8))
            pQ = ctx.enter_context(tc.tile_pool(name="pQ", bufs=8))

            id8 = consts.tile([8, 8], F32)
            make_identity(nc, id8[:])
            ones_c = consts.tile([P, 1], F32)
            nc.gpsimd.memset(ones_c[:], 1.0)

            st_spill = [dram.tile([P, SQ], F32, tag="stsp", name=f"stsp{i}")
                        for i in range(NSPILL)]

            dmae = [nc.sync, nc.scalar, nc.gpsimd]

            # ---- tile allocation in rotation order -----------------
            wqs = [pB.tile([P, D], F32R, tag="b", name=f"wq{i}")
                   for i in range(ND)]
            wks = [pC.tile([P, D], F32R, tag="c", name=f"wk{i}")
                   for i in range(ND)]
            qts = [pQ.tile([P, SQ], F32R, tag="q", name=f"qin{i}")
                   for i in range(ND)]
            ktsA = [pA.tile([P, SQ], F32R, tag="a", name=f"kta{i}")
                    for i in range(ND)]
            ktsB = [pA.tile([P, SQ], F32R, tag="a", name=f"ktb{i}")
                    for i in range(ND)]
            MT = [pD.tile([P, D], F32R, tag="d", name=f"mt{i}")
                  for i in range(ND)]

            # input loads in consumption order, round-robin 3 queues:
            # wq/wk pairs feed P0 immediately; qT lands during P0 for
            # P1; kT during P1 for P2. kraw/wv/reloads are issued later
            # (inside P2) once their victim slots are dead.
            for d in range(ND):
                dmae[(2 * d) % 3].dma_start(wqs[d][:], wq.ap()[ts(d, P), :])
                dmae[(2 * d + 1) % 3].dma_start(wks[d][:], wk.ap()[ts(d, P), :])
            for d in range(ND):
                dmae[d % 3].dma_start(qts[d][:], qT.ap()[ts(d, P), :])
            for d in range(ND):
                dmae[(2 * d) % 3].dma_start(ktsA[d][:], kT.ap()[ts(d, P), 0:SQ])
                dmae[(2 * d + 1) % 3].dma_start(ktsB[d][:], kT.ap()[ts(d, P), SQ:S])

            # ---- P0: MT[d,d'] = Wq^T Wk ----------------------------
            # e-inner over 4-d-groups: the first matmul needs only
            # wqs[0]+wks[0]; each arriving e-pair feeds 4 matmuls.
            for half in range(2):
                for dg in range(2):
                    pss = [psum.tile([P, 512], F32, tag="mm",
                                     name=f"ps_m{half}_{dg}_{i}")
                           for i in range(4)]
                    for e in range(NE):
                        for i in range(4):
                            nc.tensor.matmul(pss[i][:],
                                             wqs[e][:, ts(dg * 4 + i, P)],
                                             wks[e][:, ts(half, 512)],
                                             start=(e == 0),
                                             stop=(e == NE - 1))
                    for i in range(4):
                        nc.vector.tensor_copy(
                            MT[dg * 4 + i][:, ts(half, 512)], pss[i][:])

            # ---- P1: AT[d',q] = MT qT ------------------------------
            AT = [pC.tile([P, SQ], F32R, tag="c", name=f"at{i}")
                  for i in range(ND)]
            for qh in range(2):
                for dg in range(2):
                    pss = [psum.tile([P, 512], F32, tag="mm",
                                     name=f"ps_a{qh}_{dg}_{i}")
                           for i in range(4)]
                    for d in range(ND):
                        for i in range(4):
                            nc.tensor.matmul(pss[i][:],
                                             MT[d][:, ts(dg * 4 + i, P)],
                                             qts[d][:, ts(qh, 512)],
                                             start=(d == 0),
                                             stop=(d == ND - 1))
                    for i in range(4):
                        nc.vector.tensor_copy(
                            AT[dg * 4 + i][:, ts(qh, 512)], pss[i][:])

            # softmax scratch lands in B (wq victims, dead after P0)
            m_bc = pB.tile([P, SQ], F32, tag="b", name="m_bc")
            macc = pB.tile([P, SQ], F32, tag="b", name="macc")
            lacc = pB.tile([P, SQ], F32, tag="b", name="lacc")
            spare = pB.tile([P, SQ], F32, tag="b", name="spare")  # noqa: F841

            # ---- P2: ST[k,q] = kT^T AT; DVE row-max on the fly -----
            # k=0..7 spill to DRAM (slot reused 8 tiles later by the
            # rotation); k=8..15 stay resident in D (MT victims).
            st_tiles = {}
            krs2 = []
            for k in range(NK):
                st_k = pD.tile([P, SQ], F32, tag="d", name=f"stb{k}")
                kts = ktsA if k < 8 else ktsB
                kk = k % 8
                for qh in range(2):
                    ps = psum.tile([P, 512], F32, tag="mm",
                                   name=f"ps_s{k}_{qh}")
                    for dp in range(ND):
                        nc.tensor.matmul(ps[:], kts[dp][:, ts(kk, P)],
                                         AT[dp][:, ts(qh, 512)],
                                         start=(dp == 0),
                                         stop=(dp == ND - 1))
                    nc.vector.tensor_copy(st_k[:, ts(qh, 512)], ps[:])
                if k == 0:
                    nc.vector.tensor_copy(macc[:], st_k[:])
                else:
                    nc.vector.tensor_max(macc[:], macc[:], st_k[:])
                if k < NSPILL:
                    nc.sync.dma_start(st_spill[k][:], st_k[:])
                else:
                    st_tiles[k] = st_k
                if k == 7:
                    # ktsA is dead: issue kraw (its victim) now so the
                    # 4MB lands under P2's second half; ST reloads (qts
                    # victims, dead since P1) prefetch on gpsimd.
                    krs2 = [pA.tile([P, 2 * SQ], BF16, tag="a",
                                    name=f"kr{j}") for j in range(NK // 2)]
                    for j in range(NK // 2):
                        for h in range(2):
                            kk2 = KORDER[2 * j + h]
                            dmae[(2 * j + h) % 2].dma_start(
                                krs2[j][:, ts(h, SQ)],
                                kraw.ap()[ts(kk2, P), :])
                    for kr in range(NSPILL):
                        st_r = pQ.tile([P, SQ], F32, tag="q",
                                       name=f"rl{kr}")
                        nc.gpsimd.dma_start(st_r[:], st_spill[kr][:])
                        st_tiles[kr] = st_r

            # wv (AT victims, dead at P2 end) + UT slots in C
            wv2 = [pC.tile([P, 2 * SQ], BF16, tag="c", name=f"wv{j}")
                   for j in range(ND // 2)]
            for j in range(ND // 2):
                for h in range(2):
                    nc.sync.dma_start(
                        wv2[j][:, ts(h, SQ)], wvT.ap()[ts(2 * j + h, P), :])
            ut2 = [pC.tile([P, 2 * SQ], BF16, tag="c", name=f"ut{j}")
                   for j in range(ND // 2)]

            # ---- P3: row max across partitions, broadcast to all ---
            nc.gpsimd.partition_all_reduce(m_bc[:], macc[:], channels=P,
                                           reduce_op=bass_isa.ReduceOp.max)

            # ---- P4+P5 interleaved: E = exp(ST - m) bf16, l on DVE;
            # UT[d,q] = key^T E, 4 concurrent PSUM groups, k-outer ----
            E2 = [pA.tile([P, 2 * SQ], BF16, tag="a", name=f"e{j}")
                  for j in range(NK // 2)]

            def e_ap(i, qh):        # [P,512] E view for KORDER[i]
                return E2[i // 2][:, (i % 2) * SQ + qh * 512:
                                  (i % 2) * SQ + qh * 512 + 512]

            def kr_ap(i, dp):       # lhsT slice for KORDER[i], d-tile dp
                return krs2[i // 2][:, (i % 2) * SQ + dp * P:
                                    (i % 2) * SQ + (dp + 1) * P]

            def ut_ap(dp, c0, w_):  # [P,w_] UT view for d-tile dp
                return ut2[dp // 2][:, (dp % 2) * SQ + c0:
                                    (dp % 2) * SQ + c0 + w_]

            for qh in range(2):
                sl = ts(qh, 512)
                for i, k in enumerate(KORDER):
                    st_k = st_tiles[k]
                    nc.vector.tensor_sub(st_k[:, sl], st_k[:, sl],
                                         m_bc[:, sl])
                    nc.scalar.activation(e_ap(i, qh), st_k[:, sl],
                                         mybir.ActivationFunctionType.Exp)
                    if i == 1:
                        nc.vector.tensor_add(lacc[:, sl], e_ap(0, qh),
                                             e_ap(1, qh))
                    elif i > 1:
                        nc.vector.tensor_add(lacc[:, sl], lacc[:, sl],
                                             e_ap(i, qh))
                for dg in range(2):
                    pss = [psum.tile([P, 512], F32, tag="mm",
                                     name=f"ps_u{qh}_{dg}_{i}")
                           for i in range(4)]
                    for i in range(NK):
                        for t in range(4):
                            nc.tensor.matmul(pss[t][:],
                                             kr_ap(i, dg * 4 + t),
                                             e_ap(i, qh),
                                             start=(i == 0),
                                             stop=(i == NK - 1))
                    for t in range(4):
                        nc.vector.tensor_copy(
                            ut_ap(dg * 4 + t, qh * 512, 512), pss[t][:])

            # ---- P6: O[q,e] = UT^T Wv^T; 1/l path threaded in ------
            groups = [(qc, eh) for qc in range(NQC) for eh in range(D // 512)]
            l_row = tiny.tile([1, SQ], F32)
            r_dram = dram.tile([1, SQ], F32)
            r8 = tiny.tile([8, P], F32)
            pt8 = psl.tile([P, 8], F32, tag="pt8")
            recip_t = tiny.tile([P, 8], F32)
            pending = []

            def emit_store(qc, eh, ot, i):
                nc.vector.tensor_scalar_mul(ot[:], ot[:], recip_t[:, qc:qc + 1])
                eng = nc.sync if i % 2 == 0 else nc.scalar
                eng.dma_start(out.ap()[ts(qc, P), ts(eh, 512)], ot[:])

            def wv_ap(dp, eh):      # [P,512] Wv^T view for d-tile dp
                return wv2[dp // 2][:, (dp % 2) * SQ + eh * 512:
                                    (dp % 2) * SQ + eh * 512 + 512]

            for g, (qc, eh) in enumerate(groups):
                ps = psum.tile([P, 512], F32, tag="mm", name=f"ps_o{qc}_{eh}")
                for dp in range(ND):
                    nc.tensor.matmul(ps[:], ut_ap(dp, qc * P, P),
                                     wv_ap(dp, eh),
                                     start=(dp == 0), stop=(dp == ND - 1))
                ot = pB.tile([P, 512], F32, tag="b", name=f"ot{qc}_{eh}")
                nc.vector.tensor_copy(ot[:], ps[:])
                if g < 5:
                    pending.append((qc, eh, ot))
                else:
                    emit_store(qc, eh, ot, g)
                if g == 2:
                    for qh in range(2):
                        pl = psl.tile([1, 512], F32, tag="pl", name=f"pl{qh}")
                        nc.tensor.matmul(pl[:], ones_c[:], lacc[:, ts(qh, 512)],
                                         start=True, stop=True)
                        nc.vector.tensor_copy(l_row[0:1, ts(qh, 512)], pl[:])
                    nc.sync.dma_start(r_dram[:], l_row[:])
                    nc.sync.dma_start(
                        r8[:], r_dram[0, :].rearrange("(a b) -> a b", a=8))
                elif g == 4:
                    nc.tensor.transpose(pt8[:], r8[:], id8[:])
                    nc.vector.reciprocal(recip_t[:], pt8[:])
                    for i, (pqc, peh, pot) in enumerate(pending):
                        emit_store(pqc, peh, pot, i)

    nc.compile()
    _NC_CACHE["nc"] = nc
    return nc


def make_in_maps(query, key, Wq, Wk, Wv):
    query = np.asarray(query, dtype=np.float32)
    key = np.asarray(key, dtype=np.float32)
    wq_np = np.ascontiguousarray(np.asarray(Wq, dtype=np.float32))
    wk_np = np.ascontiguousarray(np.asarray(Wk, dtype=np.float32))
    wvT_np = np.ascontiguousarray(
        np.asarray(Wv, dtype=np.float32).T.astype(ml_dtypes.bfloat16))

    in_maps = []
    for c in range(N_CORES):
        b, h = c // 2, c % 2
        qTn = np.ascontiguousarray(query[b, h * SQ:(h + 1) * SQ, :].T)
        kTn = np.ascontiguousarray(key[b].T)
        krn = np.ascontiguousarray(key[b].astype(ml_dtypes.bfloat16))
        in_maps.append({
            "qT": qTn, "kT": kTn, "kraw": krn,
            "wq": wq_np, "wk": wk_np, "wvT": wvT_np,
        })
    return in_maps


def kernel(query, key, Wq, Wk, Wv):
    in_maps = make_in_maps(query, key, Wq, Wk, Wv)
    nc = _build()
    res = run_bass_kernel_spmd(nc, in_maps, core_ids=list(range(N_CORES)))
    outv = np.empty((B, S, D), dtype=np.float32)
    for c in range(N_CORES):
        b, h = c // 2, c % 2
        outv[b, h * SQ:(h + 1) * SQ, :] = res.results[c]["out"]
    return outv


# revision 9
# speedup vs baseline: 1.3380x; 1.0429x over previous
"""Trainium2 Bass kernel: attention layer (B=4, S=2048, D=1024), 8 NeuronCores.

Sharding: data-parallel over (batch, query-half) -> 8 shards. Each core
computes one batch's half of the queries against that batch's full keys.

Algorithm (reassociated to cut PE work ~22% vs the direct form):
  scores = (q Wq^T)(key Wk^T)^T = q (Wq^T Wk) key^T
    -> MT = Wq^T Wk (128 matmuls, needs only the two weight matrices so
       the PE starts ~2 tile-loads into the kernel), AT = MT qT (128),
       then scores contract the RAW key (kT input, f32r) with AT (256)
       instead of Q-proj(128) + K-proj(256) + scores(256).
  out = P (key Wv^T) = (P key) Wv^T
    -> contract P with the raw key first (U^T = key^T E, 256 matmuls),
       then one 1024x1024 transform (128 matmuls) instead of projecting
       V for all 2048 keys (256) + PV (256). V is never materialized.
This removes every matmul that was redundant between the two cores of a
batch pair, with zero communication. ~898 matmuls/core vs 1155.

Per-core dataflow:
  P0  MT[d,d'] = Wq^T Wk      (f32r; e-inner sweeps of 4 d-groups pace
                               the PE at the wq/wk DMA arrival rate)
  P1  AT[d',q] = MT qT        (f32r; qT arrives during P0)
  P2  ST[k,q]  = kT^T AT      (f32r; k on partitions; row-max tracked by
                               DVE max chain; first 8 k-tiles spill to
                               DRAM, 8 stay resident; after the k=7
                               chain the kraw/wv loads and ST reloads
                               are issued so every DMA lands before the
                               softmax window)
  P3  m_bc = gpsimd.partition_all_reduce(max) over macc -- replaces the
      7-step DMA-shift halving + DRAM bounce broadcast of the baseline
      (whose serial semaphore+DMA latency cost ~40us of PE idle)
  P4  E = exp(ST - m) in bf16, qh-half-major; l accumulated on DVE
  P5  UT[d,q]  = key^T E      (bf16; lhsT = raw bf16 key tiles; 4
                               concurrent PSUM groups, k-outer, so PE
                               consumption paces the exp chain)
  P6  O[q,e]   = UT^T Wv^T    (bf16; the l ones-matmuls, PE-transpose of
                               1/l via DRAM bounce, and scaled stores
                               threaded into the O loop)

SBUF is managed as flat always-open pools of uniform 4KB/partition
slots with tag-rotation reuse (victim's last read always precedes the
new tile's first write):
  A(64KB,16): kT halves     -> kraw bf16 pairs (8) -> E pairs (8)
  B(32KB, 8): Wq tiles      -> m_bc,macc,lacc,(spare) -> O out-stage
  C(32KB, 8): Wk tiles      -> AT -> wv bf16 pairs (4) + UT pairs (4)
  D(32KB, 8): MT            -> ST residents (k=8..15)
  Q(32KB, 8): qT tiles      -> ST reloads (k=0..7)
Total ~198KB of ~203 usable. float32r keeps ~1.5e-4 relative precision
on the unscaled (logit std ~34) softmax path; bf16 is fine for E and
the U/O contractions.
"""

import numpy as np
import ml_dtypes
from contextlib import ExitStack

import concourse.bass as bass
import concourse.tile as tile
from concourse import bacc, bass_isa, mybir
from concourse.bass import ts
from concourse.bass_utils import run_bass_kernel_spmd

B, S, D = 4, 2048, 1024
N_CORES = 8
SQ = S // 2            # 1024 query rows per core
P = 128                # partitions
NE = D // P            # 8 e-tiles
ND = D // P            # 8 d-tiles
NK = S // P            # 16 k-tiles
NQC = SQ // P          # 8 q-chunks
NSPILL = 8             # ST k-tiles spilled to DRAM (rest stay resident)
F32R = mybir.dt.float32r
F32 = mybir.dt.float32
BF16 = mybir.dt.bfloat16

# E/UT production+consumption order: resident ST tiles (k=8..15) first.
KORDER = list(range(NSPILL, NK)) + list(range(NSPILL))

_NC_CACHE = {}


def _build():
    if "nc" in _NC_CACHE:
        return _NC_CACHE["nc"]
    nc = bacc.Bacc("TRN2", target_bir_lowering=False, debug=False,
                   num_devices=N_CORES)

    qT = nc.dram_tensor("qT", [D, SQ], F32R, kind="ExternalInput")
    kT = nc.dram_tensor("kT", [D, S], F32R, kind="ExternalInput")
    kraw = nc.dram_tensor("kraw", [S, D], BF16, kind="ExternalInput")
    wq = nc.dram_tensor("wq", [D, D], F32R, kind="ExternalInput")
    wk = nc.dram_tensor("wk", [D, D], F32R, kind="ExternalInput")
    wvT = nc.dram_tensor("wvT", [D, D], BF16, kind="ExternalInput")
    out = nc.dram_tensor("out", [SQ, D], F32, kind="ExternalOutput")

    from concourse.masks import make_identity

    with tile.TileContext(nc) as tc:
        with ExitStack() as ctx:
            psum = ctx.enter_context(tc.tile_pool(name="psum", bufs=6, space="PSUM"))
            psl = ctx.enter_context(tc.tile_pool(name="psl", bufs=1, space="PSUM"))
            dram = ctx.enter_context(tc.tile_pool(name="dram", bufs=1, space="DRAM"))
            consts = ctx.enter_context(tc.tile_pool(name="consts", bufs=1))
            tiny = ctx.enter_context(tc.tile_pool(name="tiny", bufs=1))
            pA = ctx.enter_context(tc.tile_pool(name="pA", bufs=16))
            pB = ctx.enter_context(tc.tile_pool(name="pB", bufs=8))
            pC = ctx.enter_context(tc.tile_pool(name="pC", bufs=8))
            pD = ctx.enter_context(tc.tile_pool(name="pD", bufs=8))
            pQ = ctx.enter_context(tc.tile_pool(name="pQ", bufs=8))

            id8 = consts.tile([8, 8], F32)
            make_identity(nc, id8[:])
            ones_c = consts.tile([P, 1], F32)
            nc.gpsimd.memset(ones_c[:], 1.0)

            st_spill = [dram.tile([P, SQ], F32, tag="stsp", name=f"stsp{i}")
                        for i in range(NSPILL)]

            dmae = [nc.sync, nc.scalar, nc.gpsimd]

            # ---- tile allocation in rotation order -----------------
            wqs = [pB.tile([P, D], F32R, tag="b", name=f"wq{i}")
                   for i in range(ND)]
            wks = [pC.tile([P, D], F32R, tag="c", name=f"wk{i}")
                   for i in range(ND)]
            qts = [pQ.tile([P, SQ], F32R, tag="q", name=f"qin{i}")
                   for i in range(ND)]
            ktsA = [pA.tile([P, SQ], F32R, tag="a", name=f"kta{i}")
                    for i in range(ND)]
            ktsB = [pA.tile([P, SQ], F32R, tag="a", name=f"ktb{i}")
                    for i in range(ND)]
            MT = [pD.tile([P, D], F32R, tag="d", name=f"mt{i}")
                  for i in range(ND)]

            # input loads in consumption order, round-robin 3 queues:
            # wq/wk pairs feed P0 immediately; qT lands during P0 for
            # P1; kT during P1 for P2. kraw/wv/reloads are issued later
            # (inside P2) once their victim slots are dead.
            for d in range(ND):
                dmae[(2 * d) % 3].dma_start(wqs[d][:, 0:512],
                                            wq.ap()[ts(d, P), 0:512])
                dmae[(2 * d + 1) % 3].dma_start(wks[d][:, 0:512],
                                                wk.ap()[ts(d, P), 0:512])
            for d in range(ND):
                dmae[(2 * d) % 3].dma_start(wqs[d][:, 512:D],
                                            wq.ap()[ts(d, P), 512:D])
                dmae[(2 * d + 1) % 3].dma_start(wks[d][:, 512:D],
                                                wk.ap()[ts(d, P), 512:D])
            for d in range(ND):
                dmae[d % 3].dma_start(qts[d][:], qT.ap()[ts(d, P), :])
            for d in range(ND):
                dmae[(2 * d) % 3].dma_start(ktsA[d][:], kT.ap()[ts(d, P), 0:SQ])
                dmae[(2 * d + 1) % 3].dma_start(ktsB[d][:], kT.ap()[ts(d, P), SQ:S])

            # ---- P0: MT[d,d'] = Wq^T Wk ----------------------------
            # e-inner over 4-d-groups: the first matmul needs only
            # wqs[0]+wks[0]; each arriving e-pair feeds 4 matmuls.
            for half in range(2):
                for dg in range(2):
                    pss = [psum.tile([P, 512], F32, tag="mm",
                                     name=f"ps_m{half}_{dg}_{i}")
                           for i in range(4)]
                    for e in range(NE):
                        for i in range(4):
                            nc.tensor.matmul(pss[i][:],
                                             wqs[e][:, ts(dg * 4 + i, P)],
                                             wks[e][:, ts(half, 512)],
                                             start=(e == 0),
                                             stop=(e == NE - 1))
                    for i in range(4):
                        nc.vector.tensor_copy(
                            MT[dg * 4 + i][:, ts(half, 512)], pss[i][:])

            # ---- P1: AT[d',q] = MT qT ------------------------------
            AT = [pC.tile([P, SQ], F32R, tag="c", name=f"at{i}")
                  for i in range(ND)]
            for qh in range(2):
                for dg in range(2):
                    pss = [psum.tile([P, 512], F32, tag="mm",
                                     name=f"ps_a{qh}_{dg}_{i}")
                           for i in range(4)]
                    for d in range(ND):
                        for i in range(4):
                            nc.tensor.matmul(pss[i][:],
                                             MT[d][:, ts(dg * 4 + i, P)],
                                             qts[d][:, ts(qh, 512)],
                                             start=(d == 0),
                                             stop=(d == ND - 1))
                    for i in range(4):
                        nc.vector.tensor_copy(
                            AT[dg * 4 + i][:, ts(qh, 512)], pss[i][:])

            # softmax scratch lands in B (wq victims, dead after P0)
            m_bc = pB.tile([P, SQ], F32, tag="b", name="m_bc")
            macc = pB.tile([P, SQ], F32, tag="b", name="macc")
            lacc = pB.tile([P, SQ], F32, tag="b", name="lacc")
            spare = pB.tile([P, SQ], F32, tag="b", name="spare")  # noqa: F841

            # ---- P2: ST[k,q] = kT^T AT; DVE row-max on the fly -----
            # k=0..7 spill to DRAM (slot reused 8 tiles later by the
            # rotation); k=8..15 stay resident in D (MT victims).
            st_tiles = {}
            krs2 = []
            for k in range(NK):
                st_k = pD.tile([P, SQ], F32, tag="d", name=f"stb{k}")
                kts = ktsA if k < 8 else ktsB
                kk = k % 8
                for qh in range(2):
                    sl = ts(qh, 512)
                    ps = psum.tile([P, 512], F32, tag="mm",
                                   name=f"ps_s{k}_{qh}")
                    for dp in range(ND):
                        nc.tensor.matmul(ps[:], kts[dp][:, ts(kk, P)],
                                         AT[dp][:, ts(qh, 512)],
                                         start=(dp == 0),
                                         stop=(dp == ND - 1))
                    nc.vector.tensor_copy(st_k[:, sl], ps[:])
                    # per-half running max, so each half's partition
                    # reduce can launch the moment k=15's chain for
                    # that half drains (qh0's runs under qh1's chain)
                    if k == 0:
                        nc.vector.tensor_copy(macc[:, sl], st_k[:, sl])
                    else:
                        nc.vector.tensor_max(macc[:, sl], macc[:, sl],
                                             st_k[:, sl])
                    if k == NK - 1:
                        nc.gpsimd.partition_all_reduce(
                            m_bc[:, sl], macc[:, sl], channels=P,
                            reduce_op=bass_isa.ReduceOp.max)
                if k < NSPILL:
                    nc.sync.dma_start(st_spill[k][:], st_k[:])
                else:
                    st_tiles[k] = st_k
                if k == 7:
                    # ktsA is dead: issue kraw (its victim) now so the
                    # 4MB lands under P2's second half; ST reloads (qts
                    # victims, dead since P1) prefetch on gpsimd.
                    krs2 = [pA.tile([P, 2 * SQ], BF16, tag="a",
                                    name=f"kr{j}") for j in range(NK // 2)]
                    for j in range(NK // 2):
                        for h in range(2):
                            kk2 = KORDER[2 * j + h]
                            dmae[(2 * j + h) % 2].dma_start(
                                krs2[j][:, ts(h, SQ)],
                                kraw.ap()[ts(kk2, P), :])
                    for kr in range(NSPILL):
                        st_r = pQ.tile([P, SQ], F32, tag="q",
                                       name=f"rl{kr}")
                        nc.gpsimd.dma_start(st_r[:], st_spill[kr][:])
                        st_tiles[kr] = st_r

            # wv (AT victims, dead at P2 end) + UT slots in C
            wv2 = [pC.tile([P, 2 * SQ], BF16, tag="c", name=f"wv{j}")
                   for j in range(ND // 2)]
            for j in range(ND // 2):
                for h in range(2):
                    nc.sync.dma_start(
                        wv2[j][:, ts(h, SQ)], wvT.ap()[ts(2 * j + h, P), :])
            ut2 = [pC.tile([P, 2 * SQ], BF16, tag="c", name=f"ut{j}")
                   for j in range(ND // 2)]

            # ---- P4+P5 interleaved: E = exp(ST - m) bf16, l on DVE;
            # UT[d,q] = key^T E, 4 concurrent PSUM groups, k-outer ----
            E2 = [pA.tile([P, 2 * SQ], BF16, tag="a", name=f"e{j}")
                  for j in range(NK // 2)]

            def e_ap(i, qh):        # [P,512] E view for KORDER[i]
                return E2[i // 2][:, (i % 2) * SQ + qh * 512:
                                  (i % 2) * SQ + qh * 512 + 512]

            def kr_ap(i, dp):       # lhsT slice for KORDER[i], d-tile dp
                return krs2[i // 2][:, (i % 2) * SQ + dp * P:
                                    (i % 2) * SQ + (dp + 1) * P]

            def ut_ap(dp, c0, w_):  # [P,w_] UT view for d-tile dp
                return ut2[dp // 2][:, (dp % 2) * SQ + c0:
                                    (dp % 2) * SQ + c0 + w_]

            for qh in range(2):
                sl = ts(qh, 512)
                for i, k in enumerate(KORDER):
                    st_k = st_tiles[k]
                    nc.vector.tensor_sub(st_k[:, sl], st_k[:, sl],
                                         m_bc[:, sl])
                    nc.scalar.activation(e_ap(i, qh), st_k[:, sl],
                                         mybir.ActivationFunctionType.Exp)
                    if i == 1:
                        nc.vector.tensor_add(lacc[:, sl], e_ap(0, qh),
                                             e_ap(1, qh))
                    elif i > 1:
                        nc.vector.tensor_add(lacc[:, sl], lacc[:, sl],
                                             e_ap(i, qh))
                for dg in range(2):
                    pss = [psum.tile([P, 512], F32, tag="mm",
                                     name=f"ps_u{qh}_{dg}_{i}")
                           for i in range(4)]
                    for i in range(NK):
                        for t in range(4):
                            nc.tensor.matmul(pss[t][:],
                                             kr_ap(i, dg * 4 + t),
                                             e_ap(i, qh),
                                             start=(i == 0),
                                             stop=(i == NK - 1))
                    for t in range(4):
                        nc.vector.tensor_copy(
                            ut_ap(dg * 4 + t, qh * 512, 512), pss[t][:])

            # ---- P6: O[q,e] = UT^T Wv^T; 1/l path threaded in ------
            groups = [(qc, eh) for qc in range(NQC) for eh in range(D // 512)]
            l_row = tiny.tile([1, SQ], F32)
            r_dram = dram.tile([1, SQ], F32)
            r8 = tiny.tile([8, P], F32)
            pt8 = psl.tile([P, 8], F32, tag="pt8")
            recip_t = tiny.tile([P, 8], F32)
            pending = []

            def emit_store(qc, eh, ot, i):
                nc.vector.tensor_scalar_mul(ot[:], ot[:], recip_t[:, qc:qc + 1])
                eng = nc.sync if i % 2 == 0 else nc.scalar
                eng.dma_start(out.ap()[ts(qc, P), ts(eh, 512)], ot[:])

            def wv_ap(dp, eh):      # [P,512] Wv^T view for d-tile dp
                return wv2[dp // 2][:, (dp % 2) * SQ + eh * 512:
                                    (dp % 2) * SQ + eh * 512 + 512]

            for g, (qc, eh) in enumerate(groups):
                ps = psum.tile([P, 512], F32, tag="mm", name=f"ps_o{qc}_{eh}")
                for dp in range(ND):
                    nc.tensor.matmul(ps[:], ut_ap(dp, qc * P, P),
                                     wv_ap(dp, eh),
                                     start=(dp == 0), stop=(dp == ND - 1))
                ot = pB.tile([P, 512], F32, tag="b", name=f"ot{qc}_{eh}")
                nc.vector.tensor_copy(ot[:], ps[:])
                if g < 5:
                    pending.append((qc, eh, ot))
                else:
                    emit_store(qc, eh, ot, g)
                if g == 2:
                    for qh in range(2):
                        pl = psl.tile([1, 512], F32, tag="pl", name=f"pl{qh}")
                        nc.tensor.matmul(pl[:], ones_c[:], lacc[:, ts(qh, 512)],
                                         start=True, stop=True)
                        nc.vector.tensor_copy(l_row[0:1, ts(qh, 512)], pl[:])
                    nc.sync.dma_start(r_dram[:], l_row[:])
                    nc.sync.dma_start(
                        r8[:], r_dram[0, :].rearrange("(a b) -> a b", a=8))
                elif g == 4:
                    nc.tensor.transpose(pt8[:], r8[:], id8[:])
                    nc.vector.reciprocal(recip_t[:], pt8[:])
                    for i, (pqc, peh, pot) in enumerate(pending):
                        emit_store(pqc, peh, pot, i)

    nc.compile()
    _NC_CACHE["nc"] = nc
    return nc


def make_in_maps(query, key, Wq, Wk, Wv):
    query = np.asarray(query, dtype=np.float32)
    key = np.asarray(key, dtype=np.float32)
    wq_np = np.ascontiguousarray(np.asarray(Wq, dtype=np.float32))
    wk_np = np.ascontiguousarray(np.asarray(Wk, dtype=np.float32))
    wvT_np = np.ascontiguousarray(
        np.asarray(Wv, dtype=np.float32).T.astype(ml_dtypes.bfloat16))

    in_maps = []
    for c in range(N_CORES):
        b, h = c // 2, c % 2
        qTn = np.ascontiguousarray(query[b, h * SQ:(h + 1) * SQ, :].T)
        kTn = np.ascontiguousarray(key[b].T)
        krn = np.ascontiguousarray(key[b].astype(ml_dtypes.bfloat16))
        in_maps.append({
            "qT": qTn, "kT": kTn, "kraw": krn,
            "wq": wq_np, "wk": wk_np, "wvT": wvT_np,
        })
    return in_maps


def kernel(query, key, Wq, Wk, Wv):
    in_maps = make_in_maps(query, key, Wq, Wk, Wv)
    nc = _build()
    res = run_bass_kernel_spmd(nc, in_maps, core_ids=list(range(N_CORES)))
    outv = np.empty((B, S, D), dtype=np.float32)
    for c in range(N_CORES):
        b, h = c // 2, c % 2
        outv[b, h * SQ:(h + 1) * SQ, :] = res.results[c]["out"]
    return outv


# revision 13
# speedup vs baseline: 1.3466x; 1.0064x over previous
"""Trainium2 Bass kernel: attention layer (B=4, S=2048, D=1024), 8 NeuronCores.

Sharding: data-parallel over (batch, query-half) -> 8 shards. Each core
computes one batch's half of the queries against that batch's full keys.

Algorithm (reassociated to cut PE work ~22% vs the direct form):
  scores = (q Wq^T)(key Wk^T)^T = q (Wq^T Wk) key^T
    -> MT = Wq^T Wk (128 matmuls, needs only the two weight matrices so
       the PE starts ~2 tile-loads into the kernel), AT = MT qT (128),
       then scores contract the RAW key (kT input, f32r) with AT (256)
       instead of Q-proj(128) + K-proj(256) + scores(256).
  out = P (key Wv^T) = (P key) Wv^T
    -> contract P with the raw key first (U^T = key^T E, 256 matmuls),
       then one 1024x1024 transform (128 matmuls) instead of projecting
       V for all 2048 keys (256) + PV (256). V is never materialized.
This removes every matmul that was redundant between the two cores of a
batch pair, with zero communication. ~898 matmuls/core vs 1155.

Per-core dataflow:
  P0  MT[d,d'] = Wq^T Wk      (f32r; e-inner sweeps of 4 d-groups pace
                               the PE at the wq/wk DMA arrival rate)
  P1  AT[d',q] = MT qT        (f32r; qT arrives during P0)
  P2  ST[k,q]  = kT^T AT      (f32r; k on partitions; row-max tracked by
                               DVE max chain; first 8 k-tiles spill to
                               DRAM, 8 stay resident; after the k=7
                               chain the kraw/wv loads and ST reloads
                               are issued so every DMA lands before the
                               softmax window)
  P3  m_bc = gpsimd.partition_all_reduce(max) over macc -- replaces the
      7-step DMA-shift halving + DRAM bounce broadcast of the baseline
      (whose serial semaphore+DMA latency cost ~40us of PE idle)
  P4  E = exp(ST - m) in bf16, qh-half-major; l accumulated on DVE
  P5  UT[d,q]  = key^T E      (bf16; lhsT = raw bf16 key tiles; 4
                               concurrent PSUM groups, k-outer, so PE
                               consumption paces the exp chain)
  P6  O[q,e]   = UT^T Wv^T    (bf16; the l ones-matmuls, PE-transpose of
                               1/l via DRAM bounce, and scaled stores
                               threaded into the O loop)

SBUF is managed as flat always-open pools of uniform 4KB/partition
slots with tag-rotation reuse (victim's last read always precedes the
new tile's first write):
  A(64KB,16): kT halves     -> kraw bf16 pairs (8) -> E pairs (8)
  B(32KB, 8): Wq tiles      -> m_bc,macc,lacc,(spare) -> O out-stage
  C(32KB, 8): Wk tiles      -> AT -> wv bf16 pairs (4) + UT pairs (4)
  D(32KB, 8): MT            -> ST residents (k=8..15)
  Q(32KB, 8): qT tiles      -> ST reloads (k=0..7)
Total ~198KB of ~203 usable. float32r keeps ~1.5e-4 relative precision
on the unscaled (logit std ~34) softmax path; bf16 is fine for E and
the U/O contractions.
"""

import numpy as np
import ml_dtypes
from contextlib import ExitStack

import concourse.bass as bass
import concourse.tile as tile
from concourse import bacc, bass_isa, mybir
from concourse.bass import ts
from concourse.bass_utils import run_bass_kernel_spmd

B, S, D = 4, 2048, 1024
N_CORES = 8
SQ = S // 2            # 1024 query rows per core
P = 128                # partitions
NE = D // P            # 8 e-tiles
ND = D // P            # 8 d-tiles
NK = S // P            # 16 k-tiles
NQC = SQ // P          # 8 q-chunks
NSPILL = 8             # ST k-tiles spilled to DRAM (rest stay resident)
F32R = mybir.dt.float32r
F32 = mybir.dt.float32
BF16 = mybir.dt.bfloat16

# E/UT production+consumption order: resident ST tiles (k=8..15) first.
KORDER = list(range(NSPILL, NK)) + list(range(NSPILL))

_NC_CACHE = {}


def _build():
    if "nc" in _NC_CACHE:
        return _NC_CACHE["nc"]
    nc = bacc.Bacc("TRN2", target_bir_lowering=False, debug=False,
                   num_devices=N_CORES)

    qT = nc.dram_tensor("qT", [D, SQ], F32R, kind="ExternalInput")
    kT = nc.dram_tensor("kT", [D, S], F32R, kind="ExternalInput")
    kraw = nc.dram_tensor("kraw", [S, D], BF16, kind="ExternalInput")
    wq = nc.dram_tensor("wq", [D, D], F32R, kind="ExternalInput")
    wk = nc.dram_tensor("wk", [D, D], F32R, kind="ExternalInput")
    wvT = nc.dram_tensor("wvT", [D, D], BF16, kind="ExternalInput")
    out = nc.dram_tensor("out", [SQ, D], F32, kind="ExternalOutput")

    from concourse.masks import make_identity

    with tile.TileContext(nc) as tc:
        with ExitStack() as ctx:
            psum = ctx.enter_context(tc.tile_pool(name="psum", bufs=6, space="PSUM"))
            psl = ctx.enter_context(tc.tile_pool(name="psl", bufs=1, space="PSUM"))
            dram = ctx.enter_context(tc.tile_pool(name="dram", bufs=1, space="DRAM"))
            consts = ctx.enter_context(tc.tile_pool(name="consts", bufs=1))
            tiny = ctx.enter_context(tc.tile_pool(name="tiny", bufs=1))
            pA = ctx.enter_context(tc.tile_pool(name="pA", bufs=16))
            pB = ctx.enter_context(tc.tile_pool(name="pB", bufs=8))
            pC = ctx.enter_context(tc.tile_pool(name="pC", bufs=8))
            pD = ctx.enter_context(tc.tile_pool(name="pD", bufs=8))
            pQ = ctx.enter_context(tc.tile_pool(name="pQ", bufs=8))

            id8 = consts.tile([8, 8], F32)
            make_identity(nc, id8[:])
            ones_c = consts.tile([P, 1], F32)
            nc.gpsimd.memset(ones_c[:], 1.0)

            st_spill = [dram.tile([P, SQ], F32, tag="stsp", name=f"stsp{i}")
                        for i in range(NSPILL)]

            dmae = [nc.sync, nc.scalar, nc.gpsimd]

            # ---- tile allocation in rotation order -----------------
            wqs = [pB.tile([P, D], F32R, tag="b", name=f"wq{i}")
                   for i in range(ND)]
            wks = [pC.tile([P, D], F32R, tag="c", name=f"wk{i}")
                   for i in range(ND)]
            qts = [pQ.tile([P, SQ], F32R, tag="q", name=f"qin{i}")
                   for i in range(ND)]
            ktsA = [pA.tile([P, SQ], F32R, tag="a", name=f"kta{i}")
                    for i in range(ND)]
            ktsB = [pA.tile([P, SQ], F32R, tag="a", name=f"ktb{i}")
                    for i in range(ND)]
            MT = [pD.tile([P, D], F32R, tag="d", name=f"mt{i}")
                  for i in range(ND)]

            # input loads in consumption order, round-robin 3 queues:
            # wq/wk pairs feed P0 immediately; qT lands during P0 for
            # P1; kT during P1 for P2. kraw/wv/reloads are issued later
            # (inside P2) once their victim slots are dead.
            for d in range(ND):
                dmae[(2 * d) % 3].dma_start(wqs[d][:, 0:512],
                                            wq.ap()[ts(d, P), 0:512])
                dmae[(2 * d + 1) % 3].dma_start(wks[d][:, 0:512],
                                                wk.ap()[ts(d, P), 0:512])
            for d in range(ND):
                dmae[(2 * d) % 3].dma_start(wqs[d][:, 512:D],
                                            wq.ap()[ts(d, P), 512:D])
                dmae[(2 * d + 1) % 3].dma_start(wks[d][:, 512:D],
                                                wk.ap()[ts(d, P), 512:D])
            for d in range(ND):
                dmae[d % 3].dma_start(qts[d][:], qT.ap()[ts(d, P), :])
            for d in range(ND):
                dmae[(2 * d) % 3].dma_start(ktsA[d][:], kT.ap()[ts(d, P), 0:SQ])
                dmae[(2 * d + 1) % 3].dma_start(ktsB[d][:], kT.ap()[ts(d, P), SQ:S])

            # ---- P0: MT[d,d'] = Wq^T Wk ----------------------------
            # e-inner over 4-d-groups: the first matmul needs only
            # wqs[0]+wks[0]; each arriving e-pair feeds 4 matmuls.
            for half in range(2):
                for dg in range(2):
                    pss = [psum.tile([P, 512], F32, tag="mm",
                                     name=f"ps_m{half}_{dg}_{i}")
                           for i in range(4)]
                    for e in range(NE):
                        for i in range(4):
                            nc.tensor.matmul(pss[i][:],
                                             wqs[e][:, ts(dg * 4 + i, P)],
                                             wks[e][:, ts(half, 512)],
                                             start=(e == 0),
                                             stop=(e == NE - 1))
                    for i in range(4):
                        nc.vector.tensor_copy(
                            MT[dg * 4 + i][:, ts(half, 512)], pss[i][:])

            # ---- P1: AT[d',q] = MT qT ------------------------------
            AT = [pC.tile([P, SQ], F32R, tag="c", name=f"at{i}")
                  for i in range(ND)]
            for qh in range(2):
                for dg in range(2):
                    pss = [psum.tile([P, 512], F32, tag="mm",
                                     name=f"ps_a{qh}_{dg}_{i}")
                           for i in range(4)]
                    for d in range(ND):
                        for i in range(4):
                            nc.tensor.matmul(pss[i][:],
                                             MT[d][:, ts(dg * 4 + i, P)],
                                             qts[d][:, ts(qh, 512)],
                                             start=(d == 0),
                                             stop=(d == ND - 1))
                    for i in range(4):
                        nc.vector.tensor_copy(
                            AT[dg * 4 + i][:, ts(qh, 512)], pss[i][:])

            # softmax scratch lands in B (wq victims, dead after P0)
            m_bc = pB.tile([P, SQ], F32, tag="b", name="m_bc")
            macc = pB.tile([P, SQ], F32, tag="b", name="macc")
            lacc = pB.tile([P, SQ], F32, tag="b", name="lacc")
            spare = pB.tile([P, SQ], F32, tag="b", name="spare")  # noqa: F841

            # ---- P2: ST[k,q] = kT^T AT; DVE row-max on the fly -----
            # k=0..7 spill to DRAM (slot reused 8 tiles later by the
            # rotation); k=8..15 stay resident in D (MT victims).
            st_tiles = {}
            krs2 = []

            def st_chain(st_k, k, qh):
                # one (k, qh) score chain + drain + per-half running
                # max; each half's partition reduce launches the moment
                # the last chain for that half drains. The final two k
                # iterations are emitted half-interleaved (qh0 chains
                # of k=14,15 first) so the qh0 reduce hides entirely
                # under the remaining qh1 chains.
                sl = ts(qh, 512)
                kts = ktsA if k < 8 else ktsB
                ps = psum.tile([P, 512], F32, tag="mm", name=f"ps_s{k}_{qh}")
                for dp in range(ND):
                    nc.tensor.matmul(ps[:], kts[dp][:, ts(k % 8, P)],
                                     AT[dp][:, ts(qh, 512)],
                                     start=(dp == 0), stop=(dp == ND - 1))
                nc.vector.tensor_copy(st_k[:, sl], ps[:])
                if k == 0:
                    nc.vector.tensor_copy(macc[:, sl], st_k[:, sl])
                else:
                    nc.vector.tensor_max(macc[:, sl], macc[:, sl],
                                         st_k[:, sl])
                if k == NK - 1:
                    nc.gpsimd.partition_all_reduce(
                        m_bc[:, sl], macc[:, sl], channels=P,
                        reduce_op=bass_isa.ReduceOp.max)

            for k in range(NK - 2):
                st_k = pD.tile([P, SQ], F32, tag="d", name=f"stb{k}")
                for qh in range(2):
                    st_chain(st_k, k, qh)
                if k < NSPILL:
                    nc.sync.dma_start(st_spill[k][:], st_k[:])
                else:
                    st_tiles[k] = st_k
                if k == 7:
                    # ktsA is dead: issue kraw (its victim) now so the
                    # 4MB lands under P2's second half; ST reloads (qts
                    # victims, dead since P1) prefetch on gpsimd.
                    krs2 = [pA.tile([P, 2 * SQ], BF16, tag="a",
                                    name=f"kr{j}") for j in range(NK // 2)]
                    for j in range(NK // 2):
                        for h in range(2):
                            kk2 = KORDER[2 * j + h]
                            dmae[(2 * j + h) % 2].dma_start(
                                krs2[j][:, ts(h, SQ)],
                                kraw.ap()[ts(kk2, P), :])
                    for kr in range(NSPILL):
                        st_r = pQ.tile([P, SQ], F32, tag="q",
                                       name=f"rl{kr}")
                        nc.gpsimd.dma_start(st_r[:], st_spill[kr][:])
                        st_tiles[kr] = st_r
            st14 = pD.tile([P, SQ], F32, tag="d", name="stb14")
            st15 = pD.tile([P, SQ], F32, tag="d", name="stb15")
            st_tiles[NK - 2], st_tiles[NK - 1] = st14, st15
            st_chain(st14, NK - 2, 0)
            st_chain(st15, NK - 1, 0)
            st_chain(st14, NK - 2, 1)
            st_chain(st15, NK - 1, 1)

            # wv (AT victims, dead at P2 end) + UT slots in C
            wv2 = [pC.tile([P, 2 * SQ], BF16, tag="c", name=f"wv{j}")
                   for j in range(ND // 2)]
            for j in range(ND // 2):
                for h in range(2):
                    nc.sync.dma_start(
                        wv2[j][:, ts(h, SQ)], wvT.ap()[ts(2 * j + h, P), :])
            ut2 = [pC.tile([P, 2 * SQ], BF16, tag="c", name=f"ut{j}")
                   for j in range(ND // 2)]

            # ---- P4+P5 interleaved: E = exp(ST - m) bf16, l on DVE;
            # UT[d,q] = key^T E, 4 concurrent PSUM groups, k-outer ----
            E2 = [pA.tile([P, 2 * SQ], BF16, tag="a", name=f"e{j}")
                  for j in range(NK // 2)]

            def e_ap(i, qh):        # [P,512] E view for KORDER[i]
                return E2[i // 2][:, (i % 2) * SQ + qh * 512:
                                  (i % 2) * SQ + qh * 512 + 512]

            def kr_ap(i, dp):       # lhsT slice for KORDER[i], d-tile dp
                return krs2[i // 2][:, (i % 2) * SQ + dp * P:
                                    (i % 2) * SQ + (dp + 1) * P]

            def ut_ap(dp, c0, w_):  # [P,w_] UT view for d-tile dp
                return ut2[dp // 2][:, (dp % 2) * SQ + c0:
                                    (dp % 2) * SQ + c0 + w_]

            for qh in range(2):
                sl = ts(qh, 512)
                for i, k in enumerate(KORDER):
                    st_k = st_tiles[k]
                    nc.vector.tensor_sub(st_k[:, sl], st_k[:, sl],
                                         m_bc[:, sl])
                    nc.scalar.activation(e_ap(i, qh), st_k[:, sl],
                                         mybir.ActivationFunctionType.Exp)
                    if i == 1:
                        nc.vector.tensor_add(lacc[:, sl], e_ap(0, qh),
                                             e_ap(1, qh))
                    elif i > 1:
                        nc.vector.tensor_add(lacc[:, sl], lacc[:, sl],
                                             e_ap(i, qh))
                for dg in range(2):
                    pss = [psum.tile([P, 512], F32, tag="mm",
                                     name=f"ps_u{qh}_{dg}_{i}")
                           for i in range(4)]
                    for i in range(NK):
                        for t in range(4):
                            nc.tensor.matmul(pss[t][:],
                                             kr_ap(i, dg * 4 + t),
                                             e_ap(i, qh),
                                             start=(i == 0),
                                             stop=(i == NK - 1))
                    for t in range(4):
                        nc.vector.tensor_copy(
                            ut_ap(dg * 4 + t, qh * 512, 512), pss[t][:])

            # ---- P6: O[q,e] = UT^T Wv^T; 1/l path threaded in ------
            groups = [(qc, eh) for qc in range(NQC) for eh in range(D // 512)]
            l_row = tiny.tile([1, SQ], F32)
            r_dram = dram.tile([1, SQ], F32)
            r8 = tiny.tile([8, P], F32)
            pt8 = psl.tile([P, 8], F32, tag="pt8")
            recip_t = tiny.tile([P, 8], F32)
            pending = []

            def emit_store(qc, eh, ot, i):
                nc.vector.tensor_scalar_mul(ot[:], ot[:], recip_t[:, qc:qc + 1])
                eng = nc.sync if i % 2 == 0 else nc.scalar
                eng.dma_start(out.ap()[ts(qc, P), ts(eh, 512)], ot[:])

            def wv_ap(dp, eh):      # [P,512] Wv^T view for d-tile dp
                return wv2[dp // 2][:, (dp % 2) * SQ + eh * 512:
                                    (dp % 2) * SQ + eh * 512 + 512]

            for g, (qc, eh) in enumerate(groups):
                ps = psum.tile([P, 512], F32, tag="mm", name=f"ps_o{qc}_{eh}")
                for dp in range(ND):
                    nc.tensor.matmul(ps[:], ut_ap(dp, qc * P, P),
                                     wv_ap(dp, eh),
                                     start=(dp == 0), stop=(dp == ND - 1))
                ot = pB.tile([P, 512], F32, tag="b", name=f"ot{qc}_{eh}")
                nc.vector.tensor_copy(ot[:], ps[:])
                if g < 5:
                    pending.append((qc, eh, ot))
                else:
                    emit_store(qc, eh, ot, g)
                if g == 2:
                    for qh in range(2):
                        pl = psl.tile([1, 512], F32, tag="pl", name=f"pl{qh}")
                        nc.tensor.matmul(pl[:], ones_c[:], lacc[:, ts(qh, 512)],
                                         start=True, stop=True)
                        nc.vector.tensor_copy(l_row[0:1, ts(qh, 512)], pl[:])
                    nc.sync.dma_start(r_dram[:], l_row[:])
                    nc.sync.dma_start(
                        r8[:], r_dram[0, :].rearrange("(a b) -> a b", a=8))
                elif g == 4:
                    nc.tensor.transpose(pt8[:], r8[:], id8[:])
                    nc.vector.reciprocal(recip_t[:], pt8[:])
                    for i, (pqc, peh, pot) in enumerate(pending):
                        emit_store(pqc, peh, pot, i)

    nc.compile()
    _NC_CACHE["nc"] = nc
    return nc


def make_in_maps(query, key, Wq, Wk, Wv):
    query = np.asarray(query, dtype=np.float32)
    key = np.asarray(key, dtype=np.float32)
    wq_np = np.ascontiguousarray(np.asarray(Wq, dtype=np.float32))
    wk_np = np.ascontiguousarray(np.asarray(Wk, dtype=np.float32))
    wvT_np = np.ascontiguousarray(
        np.asarray(Wv, dtype=np.float32).T.astype(ml_dtypes.bfloat16))

    in_maps = []
    for c in range(N_CORES):
        b, h = c // 2, c % 2
        qTn = np.ascontiguousarray(query[b, h * SQ:(h + 1) * SQ, :].T)
        kTn = np.ascontiguousarray(key[b].T)
        krn = np.ascontiguousarray(key[b].astype(ml_dtypes.bfloat16))
        in_maps.append({
            "qT": qTn, "kT": kTn, "kraw": krn,
            "wq": wq_np, "wk": wk_np, "wvT": wvT_np,
        })
    return in_maps


def kernel(query, key, Wq, Wk, Wv):
    in_maps = make_in_maps(query, key, Wq, Wk, Wv)
    nc = _build()
    res = run_bass_kernel_spmd(nc, in_maps, core_ids=list(range(N_CORES)))
    outv = np.empty((B, S, D), dtype=np.float32)
    for c in range(N_CORES):
        b, h = c // 2, c % 2
        outv[b, h * SQ:(h + 1) * SQ, :] = res.results[c]["out"]
    return outv


# revision 17
# speedup vs baseline: 1.3551x; 1.0064x over previous
"""Trainium2 Bass kernel: attention layer (B=4, S=2048, D=1024), 8 NeuronCores.

Sharding: data-parallel over (batch, query-half) -> 8 shards. Each core
computes one batch's half of the queries against that batch's full keys.

Algorithm (reassociated to cut PE work ~22% vs the direct form):
  scores = (q Wq^T)(key Wk^T)^T = q (Wq^T Wk) key^T
    -> MT = Wq^T Wk (128 matmuls, needs only the two weight matrices so
       the PE starts ~2 tile-loads into the kernel), AT = MT qT (128),
       then scores contract the RAW key (kT input, f32r) with AT (256)
       instead of Q-proj(128) + K-proj(256) + scores(256).
  out = P (key Wv^T) = (P key) Wv^T
    -> contract P with the raw key first (U^T = key^T E, 256 matmuls),
       then one 1024x1024 transform (128 matmuls) instead of projecting
       V for all 2048 keys (256) + PV (256). V is never materialized.
This removes every matmul that was redundant between the two cores of a
batch pair, with zero communication. ~898 matmuls/core vs 1155.

Per-core dataflow:
  P0  MT[d,d'] = Wq^T Wk      (f32r; e-inner sweeps of 4 d-groups pace
                               the PE at the wq/wk DMA arrival rate)
  P1  AT[d',q] = MT qT        (f32r; qT arrives during P0)
  P2  ST[k,q]  = kT^T AT      (f32r; k on partitions; row-max tracked by
                               DVE max chain; first 8 k-tiles spill to
                               DRAM, 8 stay resident; after the k=7
                               chain the kraw/wv loads and ST reloads
                               are issued so every DMA lands before the
                               softmax window)
  P3  m_bc = gpsimd.partition_all_reduce(max) over macc -- replaces the
      7-step DMA-shift halving + DRAM bounce broadcast of the baseline
      (whose serial semaphore+DMA latency cost ~40us of PE idle)
  P4  E = exp(ST - m) in bf16, qh-half-major; l accumulated on DVE
  P5  UT[d,q]  = key^T E      (bf16; lhsT = raw bf16 key tiles; 4
                               concurrent PSUM groups, k-outer, so PE
                               consumption paces the exp chain)
  P6  O[q,e]   = UT^T Wv^T    (bf16; the l ones-matmuls, PE-transpose of
                               1/l via DRAM bounce, and scaled stores
                               threaded into the O loop)

SBUF is managed as flat always-open pools of uniform 4KB/partition
slots with tag-rotation reuse (victim's last read always precedes the
new tile's first write):
  A(64KB,16): kT halves     -> kraw bf16 pairs (8) -> E pairs (8)
  B(32KB, 8): Wq tiles      -> m_bc,macc,lacc,(spare) -> O out-stage
  C(32KB, 8): Wk tiles      -> AT -> wv bf16 pairs (4) + UT pairs (4)
  D(32KB, 8): MT            -> ST residents (k=8..15)
  Q(32KB, 8): qT tiles      -> ST reloads (k=0..7)
Total ~198KB of ~203 usable. float32r keeps ~1.5e-4 relative precision
on the unscaled (logit std ~34) softmax path; bf16 is fine for E and
the U/O contractions.
"""

import numpy as np
import ml_dtypes
from contextlib import ExitStack

import concourse.bass as bass
import concourse.tile as tile
from concourse import bacc, bass_isa, mybir
from concourse.bass import ts
from concourse.bass_utils import run_bass_kernel_spmd

B, S, D = 4, 2048, 1024
N_CORES = 8
SQ = S // 2            # 1024 query rows per core
P = 128                # partitions
NE = D // P            # 8 e-tiles
ND = D // P            # 8 d-tiles
NK = S // P            # 16 k-tiles
NQC = SQ // P          # 8 q-chunks
NSPILL = 8             # ST k-tiles spilled to DRAM (rest stay resident)
F32R = mybir.dt.float32r
F32 = mybir.dt.float32
BF16 = mybir.dt.bfloat16

# E/UT production+consumption order: resident ST tiles (k=8..15) first.
KORDER = list(range(NSPILL, NK)) + list(range(NSPILL))

_NC_CACHE = {}


def _build():
    if "nc" in _NC_CACHE:
        return _NC_CACHE["nc"]
    nc = bacc.Bacc("TRN2", target_bir_lowering=False, debug=False,
                   num_devices=N_CORES)

    qT = nc.dram_tensor("qT", [D, SQ], F32R, kind="ExternalInput")
    kT = nc.dram_tensor("kT", [D, S], F32R, kind="ExternalInput")
    kraw = nc.dram_tensor("kraw", [S, D], BF16, kind="ExternalInput")
    wq = nc.dram_tensor("wq", [D, D], F32R, kind="ExternalInput")
    wk = nc.dram_tensor("wk", [D, D], F32R, kind="ExternalInput")
    wvT = nc.dram_tensor("wvT", [D, D], BF16, kind="ExternalInput")
    out = nc.dram_tensor("out", [SQ, D], F32, kind="ExternalOutput")

    from concourse.masks import make_identity

    with tile.TileContext(nc) as tc:
        with ExitStack() as ctx:
            psum = ctx.enter_context(tc.tile_pool(name="psum", bufs=6, space="PSUM"))
            psl = ctx.enter_context(tc.tile_pool(name="psl", bufs=1, space="PSUM"))
            dram = ctx.enter_context(tc.tile_pool(name="dram", bufs=1, space="DRAM"))
            consts = ctx.enter_context(tc.tile_pool(name="consts", bufs=1))
            tiny = ctx.enter_context(tc.tile_pool(name="tiny", bufs=1))
            pA = ctx.enter_context(tc.tile_pool(name="pA", bufs=16))
            pB = ctx.enter_context(tc.tile_pool(name="pB", bufs=8))
            pC = ctx.enter_context(tc.tile_pool(name="pC", bufs=8))
            pD = ctx.enter_context(tc.tile_pool(name="pD", bufs=8))
            pQ = ctx.enter_context(tc.tile_pool(name="pQ", bufs=8))

            id8 = consts.tile([8, 8], F32)
            make_identity(nc, id8[:])
            ones_c = consts.tile([P, 1], F32)
            nc.gpsimd.memset(ones_c[:], 1.0)

            st_spill = [dram.tile([P, SQ], F32, tag="stsp", name=f"stsp{i}")
                        for i in range(NSPILL)]

            dmae = [nc.sync, nc.scalar, nc.gpsimd]

            # ---- tile allocation in rotation order -----------------
            # wq/wk live as separate lo/hi column-half tiles (packed 2
            # per 4KB slot): wave-2 DMA writes then land in different
            # slots than the halves the P0 sweeps are reading, avoiding
            # SBUF read/write bank conflicts (measured ~6us of 300-430ns
            # matmuls when a single tile was half-read, half-written).
            wqlo = [pB.tile([P, D], F32R, tag="b", name=f"wqlo{j}")
                    for j in range(4)]
            wqhi = [pB.tile([P, D], F32R, tag="b", name=f"wqhi{j}")
                    for j in range(4)]
            wklo = [pC.tile([P, D], F32R, tag="c", name=f"wklo{j}")
                    for j in range(4)]
            wkhi = [pC.tile([P, D], F32R, tag="c", name=f"wkhi{j}")
                    for j in range(4)]

            def wq_ap(e, c0, w_):   # wq[e-tile][:, c0:c0+w_], c0 half-aligned
                src = wqlo if c0 < 512 else wqhi
                base = (e % 2) * 512 + (c0 % 512)
                return src[e // 2][:, base:base + w_]

            def wk_ap(e, c0, w_):
                src = wklo if c0 < 512 else wkhi
                base = (e % 2) * 512 + (c0 % 512)
                return src[e // 2][:, base:base + w_]
            qts = [pQ.tile([P, SQ], F32R, tag="q", name=f"qin{i}")
                   for i in range(ND)]
            ktsA = [pA.tile([P, SQ], F32R, tag="a", name=f"kta{i}")
                    for i in range(ND)]
            ktsB = [pA.tile([P, SQ], F32R, tag="a", name=f"ktb{i}")
                    for i in range(ND)]
            MT = [pD.tile([P, D], F32R, tag="d", name=f"mt{i}")
                  for i in range(ND)]

            # input loads in consumption order, round-robin 3 queues:
            # wq/wk pairs feed P0 immediately; qT lands during P0 for
            # P1; kT during P1 for P2. kraw/wv/reloads are issued later
            # (inside P2) once their victim slots are dead.
            for d in range(ND):
                dmae[(2 * d) % 3].dma_start(wq_ap(d, 0, 512),
                                            wq.ap()[ts(d, P), 0:512])
                dmae[(2 * d + 1) % 3].dma_start(wk_ap(d, 0, 512),
                                                wk.ap()[ts(d, P), 0:512])
            for d in range(ND):
                dmae[(2 * d) % 3].dma_start(wq_ap(d, 512, 512),
                                            wq.ap()[ts(d, P), 512:D])
                dmae[(2 * d + 1) % 3].dma_start(wk_ap(d, 512, 512),
                                                wk.ap()[ts(d, P), 512:D])
            for d in range(ND):
                dmae[d % 3].dma_start(qts[d][:], qT.ap()[ts(d, P), :])
            for d in range(ND):
                dmae[(2 * d) % 3].dma_start(ktsA[d][:], kT.ap()[ts(d, P), 0:SQ])
                dmae[(2 * d + 1) % 3].dma_start(ktsB[d][:], kT.ap()[ts(d, P), SQ:S])

            # ---- P0: MT[d,d'] = Wq^T Wk ----------------------------
            # e-inner over 4-d-groups: the first matmul needs only
            # wqs[0]+wks[0]; each arriving e-pair feeds 4 matmuls.
            for half in range(2):
                for dg in range(2):
                    pss = [psum.tile([P, 512], F32, tag="mm",
                                     name=f"ps_m{half}_{dg}_{i}")
                           for i in range(4)]
                    for e in range(NE):
                        for i in range(4):
                            nc.tensor.matmul(pss[i][:],
                                             wq_ap(e, (dg * 4 + i) * P, P),
                                             wk_ap(e, half * 512, 512),
                                             start=(e == 0),
                                             stop=(e == NE - 1))
                    for i in range(4):
                        nc.vector.tensor_copy(
                            MT[dg * 4 + i][:, ts(half, 512)], pss[i][:])

            # ---- P1: AT[d',q] = MT qT ------------------------------
            AT = [pC.tile([P, SQ], F32R, tag="c", name=f"at{i}")
                  for i in range(ND)]
            for qh in range(2):
                for dg in range(2):
                    pss = [psum.tile([P, 512], F32, tag="mm",
                                     name=f"ps_a{qh}_{dg}_{i}")
                           for i in range(4)]
                    for d in range(ND):
                        for i in range(4):
                            nc.tensor.matmul(pss[i][:],
                                             MT[d][:, ts(dg * 4 + i, P)],
                                             qts[d][:, ts(qh, 512)],
                                             start=(d == 0),
                                             stop=(d == ND - 1))
                    for i in range(4):
                        nc.vector.tensor_copy(
                            AT[dg * 4 + i][:, ts(qh, 512)], pss[i][:])

            # softmax scratch lands in B (wq victims, dead after P0)
            m_bc = pB.tile([P, SQ], F32, tag="b", name="m_bc")
            macc = pB.tile([P, SQ], F32, tag="b", name="macc")
            lacc = pB.tile([P, SQ], F32, tag="b", name="lacc")
            spare = pB.tile([P, SQ], F32, tag="b", name="spare")  # noqa: F841

            # ---- P2: ST[k,q] = kT^T AT; DVE row-max on the fly -----
            # k=0..7 spill to DRAM (slot reused 8 tiles later by the
            # rotation); k=8..15 stay resident in D (MT victims).
            st_tiles = {}
            krs2 = []

            def st_chain(st_k, k, qh):
                # one (k, qh) score chain + drain + per-half running
                # max; each half's partition reduce launches the moment
                # the last chain for that half drains. The final two k
                # iterations are emitted half-interleaved (qh0 chains
                # of k=14,15 first) so the qh0 reduce hides entirely
                # under the remaining qh1 chains.
                sl = ts(qh, 512)
                kts = ktsA if k < 8 else ktsB
                ps = psum.tile([P, 512], F32, tag="mm", name=f"ps_s{k}_{qh}")
                for dp in range(ND):
                    nc.tensor.matmul(ps[:], kts[dp][:, ts(k % 8, P)],
                                     AT[dp][:, ts(qh, 512)],
                                     start=(dp == 0), stop=(dp == ND - 1))
                nc.vector.tensor_copy(st_k[:, sl], ps[:])
                if k == 0:
                    nc.vector.tensor_copy(macc[:, sl], st_k[:, sl])
                else:
                    nc.vector.tensor_max(macc[:, sl], macc[:, sl],
                                         st_k[:, sl])
                if k == NK - 1:
                    nc.gpsimd.partition_all_reduce(
                        m_bc[:, sl], macc[:, sl], channels=P,
                        reduce_op=bass_isa.ReduceOp.max)

            for k in range(NK - 2):
                st_k = pD.tile([P, SQ], F32, tag="d", name=f"stb{k}")
                for qh in range(2):
                    st_chain(st_k, k, qh)
                if k < NSPILL:
                    nc.sync.dma_start(st_spill[k][:], st_k[:])
                else:
                    st_tiles[k] = st_k
                if k == 7:
                    # ktsA is dead: issue kraw (its victim) now so the
                    # 4MB lands under P2's second half; ST reloads (qts
                    # victims, dead since P1) prefetch on gpsimd.
                    krs2 = [pA.tile([P, 2 * SQ], BF16, tag="a",
                                    name=f"kr{j}") for j in range(NK // 2)]
                    for j in range(NK // 2):
                        for h in range(2):
                            kk2 = KORDER[2 * j + h]
                            dmae[(2 * j + h) % 2].dma_start(
                                krs2[j][:, ts(h, SQ)],
                                kraw.ap()[ts(kk2, P), :])
                    for kr in range(NSPILL):
                        st_r = pQ.tile([P, SQ], F32, tag="q",
                                       name=f"rl{kr}")
                        nc.gpsimd.dma_start(st_r[:], st_spill[kr][:])
                        st_tiles[kr] = st_r
            st14 = pD.tile([P, SQ], F32, tag="d", name="stb14")
            st15 = pD.tile([P, SQ], F32, tag="d", name="stb15")
            st_tiles[NK - 2], st_tiles[NK - 1] = st14, st15
            st_chain(st14, NK - 2, 0)
            st_chain(st15, NK - 1, 0)
            st_chain(st14, NK - 2, 1)
            st_chain(st15, NK - 1, 1)

            # wv (AT victims, dead at P2 end) + UT slots in C
            wv2 = [pC.tile([P, 2 * SQ], BF16, tag="c", name=f"wv{j}")
                   for j in range(ND // 2)]
            for j in range(ND // 2):
                for h in range(2):
                    nc.sync.dma_start(
                        wv2[j][:, ts(h, SQ)], wvT.ap()[ts(2 * j + h, P), :])
            ut2 = [pC.tile([P, 2 * SQ], BF16, tag="c", name=f"ut{j}")
                   for j in range(ND // 2)]

            # ---- P4+P5 interleaved: E = exp(ST - m) bf16, l on DVE;
            # UT[d,q] = key^T E, 4 concurrent PSUM groups, k-outer ----
            E2 = [pA.tile([P, 2 * SQ], BF16, tag="a", name=f"e{j}")
                  for j in range(NK // 2)]

            def e_ap(i, qh):        # [P,512] E view for KORDER[i]
                return E2[i // 2][:, (i % 2) * SQ + qh * 512:
                                  (i % 2) * SQ + qh * 512 + 512]

            def kr_ap(i, dp):       # lhsT slice for KORDER[i], d-tile dp
                return krs2[i // 2][:, (i % 2) * SQ + dp * P:
                                    (i % 2) * SQ + (dp + 1) * P]

            def ut_ap(dp, c0, w_):  # [P,w_] UT view for d-tile dp
                return ut2[dp // 2][:, (dp % 2) * SQ + c0:
                                    (dp % 2) * SQ + c0 + w_]

            for qh in range(2):
                sl = ts(qh, 512)
                for i, k in enumerate(KORDER):
                    st_k = st_tiles[k]
                    nc.vector.tensor_sub(st_k[:, sl], st_k[:, sl],
                                         m_bc[:, sl])
                    nc.scalar.activation(e_ap(i, qh), st_k[:, sl],
                                         mybir.ActivationFunctionType.Exp)
                    if i == 1:
                        nc.vector.tensor_add(lacc[:, sl], e_ap(0, qh),
                                             e_ap(1, qh))
                    elif i > 1:
                        nc.vector.tensor_add(lacc[:, sl], lacc[:, sl],
                                             e_ap(i, qh))
                # 6-group then 2-group sweeps: the wider first sweep
                # consumes each E tile 6x (1.4us) vs the 0.7us exp
                # cadence, so the PE rides out the exp-chain warmup.
                for d0, gw in ((0, 6), (6, 2)):
                    pss = [psum.tile([P, 512], F32, tag="mm",
                                     name=f"ps_u{qh}_{d0}_{i}")
                           for i in range(gw)]
                    for i in range(NK):
                        for t in range(gw):
                            nc.tensor.matmul(pss[t][:],
                                             kr_ap(i, d0 + t),
                                             e_ap(i, qh),
                                             start=(i == 0),
                                             stop=(i == NK - 1))
                    for t in range(gw):
                        nc.vector.tensor_copy(
                            ut_ap(d0 + t, qh * 512, 512), pss[t][:])

            # ---- P6: O[q,e] = UT^T Wv^T; 1/l path threaded in ------
            groups = [(qc, eh) for qc in range(NQC) for eh in range(D // 512)]
            l_row = tiny.tile([1, SQ], F32)
            r_dram = dram.tile([1, SQ], F32)
            r8 = tiny.tile([8, P], F32)
            pt8 = psl.tile([P, 8], F32, tag="pt8")
            recip_t = tiny.tile([P, 8], F32)
            pending = []

            def emit_store(qc, eh, ot, i):
                nc.vector.tensor_scalar_mul(ot[:], ot[:], recip_t[:, qc:qc + 1])
                eng = nc.sync if i % 2 == 0 else nc.scalar
                eng.dma_start(out.ap()[ts(qc, P), ts(eh, 512)], ot[:])

            def wv_ap(dp, eh):      # [P,512] Wv^T view for d-tile dp
                return wv2[dp // 2][:, (dp % 2) * SQ + eh * 512:
                                    (dp % 2) * SQ + eh * 512 + 512]

            for g, (qc, eh) in enumerate(groups):
                ps = psum.tile([P, 512], F32, tag="mm", name=f"ps_o{qc}_{eh}")
                for dp in range(ND):
                    nc.tensor.matmul(ps[:], ut_ap(dp, qc * P, P),
                                     wv_ap(dp, eh),
                                     start=(dp == 0), stop=(dp == ND - 1))
                ot = pB.tile([P, 512], F32, tag="b", name=f"ot{qc}_{eh}")
                nc.vector.tensor_copy(ot[:], ps[:])
                if g < 5:
                    pending.append((qc, eh, ot))
                else:
                    emit_store(qc, eh, ot, g)
                if g == 2:
                    for qh in range(2):
                        pl = psl.tile([1, 512], F32, tag="pl", name=f"pl{qh}")
                        nc.tensor.matmul(pl[:], ones_c[:], lacc[:, ts(qh, 512)],
                                         start=True, stop=True)
                        nc.vector.tensor_copy(l_row[0:1, ts(qh, 512)], pl[:])
                    nc.sync.dma_start(r_dram[:], l_row[:])
                    nc.sync.dma_start(
                        r8[:], r_dram[0, :].rearrange("(a b) -> a b", a=8))
                elif g == 4:
                    nc.tensor.transpose(pt8[:], r8[:], id8[:])
                    nc.vector.reciprocal(recip_t[:], pt8[:])
                    for i, (pqc, peh, pot) in enumerate(pending):
                        emit_store(pqc, peh, pot, i)

    nc.compile()
    _NC_CACHE["nc"] = nc
    return nc


def make_in_maps(query, key, Wq, Wk, Wv):
    query = np.asarray(query, dtype=np.float32)
    key = np.asarray(key, dtype=np.float32)
    wq_np = np.ascontiguousarray(np.asarray(Wq, dtype=np.float32))
    wk_np = np.ascontiguousarray(np.asarray(Wk, dtype=np.float32))
    wvT_np = np.ascontiguousarray(
        np.asarray(Wv, dtype=np.float32).T.astype(ml_dtypes.bfloat16))

    in_maps = []
    for c in range(N_CORES):
        b, h = c // 2, c % 2
        qTn = np.ascontiguousarray(query[b, h * SQ:(h + 1) * SQ, :].T)
        kTn = np.ascontiguousarray(key[b].T)
        krn = np.ascontiguousarray(key[b].astype(ml_dtypes.bfloat16))
        in_maps.append({
            "qT": qTn, "kT": kTn, "kraw": krn,
            "wq": wq_np, "wk": wk_np, "wvT": wvT_np,
        })
    return in_maps


def kernel(query, key, Wq, Wk, Wv):
    in_maps = make_in_maps(query, key, Wq, Wk, Wv)
    nc = _build()
    res = run_bass_kernel_spmd(nc, in_maps, core_ids=list(range(N_CORES)))
    outv = np.empty((B, S, D), dtype=np.float32)
    for c in range(N_CORES):
        b, h = c // 2, c % 2
        outv[b, h * SQ:(h + 1) * SQ, :] = res.results[c]["out"]
    return outv


# revision 19
# speedup vs baseline: 1.3656x; 1.0078x over previous
"""Trainium2 Bass kernel: attention layer (B=4, S=2048, D=1024), 8 NeuronCores.

Sharding: data-parallel over (batch, query-half) -> 8 shards. Each core
computes one batch's half of the queries against that batch's full keys.

Algorithm (reassociated to cut PE work ~22% vs the direct form):
  scores = (q Wq^T)(key Wk^T)^T = q (Wq^T Wk) key^T
    -> MT = Wq^T Wk (128 matmuls, needs only the two weight matrices so
       the PE starts ~2 tile-loads into the kernel), AT = MT qT (128),
       then scores contract the RAW key (kT input, f32r) with AT (256)
       instead of Q-proj(128) + K-proj(256) + scores(256).
  out = P (key Wv^T) = (P key) Wv^T
    -> contract P with the raw key first (U^T = key^T E, 256 matmuls),
       then one 1024x1024 transform (128 matmuls) instead of projecting
       V for all 2048 keys (256) + PV (256). V is never materialized.
This removes every matmul that was redundant between the two cores of a
batch pair, with zero communication. ~898 matmuls/core vs 1155.

Per-core dataflow:
  P0  MT[d,d'] = Wq^T Wk      (f32r; e-inner sweeps of 4 d-groups pace
                               the PE at the wq/wk DMA arrival rate)
  P1  AT[d',q] = MT qT        (f32r; qT arrives during P0)
  P2  ST[k,q]  = kT^T AT      (f32r; k on partitions; row-max tracked by
                               DVE max chain; first 8 k-tiles spill to
                               DRAM, 8 stay resident; after the k=7
                               chain the kraw/wv loads and ST reloads
                               are issued so every DMA lands before the
                               softmax window)
  P3  m_bc = gpsimd.partition_all_reduce(max) over macc -- replaces the
      7-step DMA-shift halving + DRAM bounce broadcast of the baseline
      (whose serial semaphore+DMA latency cost ~40us of PE idle)
  P4  E = exp(ST - m) in bf16, qh-half-major; l accumulated on DVE
  P5  UT[d,q]  = key^T E      (bf16; lhsT = raw bf16 key tiles; 4
                               concurrent PSUM groups, k-outer, so PE
                               consumption paces the exp chain)
  P6  O[q,e]   = UT^T Wv^T    (bf16; the l ones-matmuls, PE-transpose of
                               1/l via DRAM bounce, and scaled stores
                               threaded into the O loop)

SBUF is managed as flat always-open pools of uniform 4KB/partition
slots with tag-rotation reuse (victim's last read always precedes the
new tile's first write):
  A(64KB,16): kT halves     -> kraw bf16 pairs (8) -> E pairs (8)
  B(32KB, 8): Wq tiles      -> m_bc,macc,lacc,(spare) -> O out-stage
  C(32KB, 8): Wk tiles      -> AT -> wv bf16 pairs (4) + UT pairs (4)
  D(32KB, 8): MT            -> ST residents (k=8..15)
  Q(32KB, 8): qT tiles      -> ST reloads (k=0..7)
Total ~198KB of ~203 usable. float32r keeps ~1.5e-4 relative precision
on the unscaled (logit std ~34) softmax path; bf16 is fine for E and
the U/O contractions.
"""

import numpy as np
import ml_dtypes
from contextlib import ExitStack

import concourse.bass as bass
import concourse.tile as tile
from concourse import bacc, bass_isa, mybir
from concourse.bass import ts
from concourse.bass_utils import run_bass_kernel_spmd

B, S, D = 4, 2048, 1024
N_CORES = 8
SQ = S // 2            # 1024 query rows per core
P = 128                # partitions
NE = D // P            # 8 e-tiles
ND = D // P            # 8 d-tiles
NK = S // P            # 16 k-tiles
NQC = SQ // P          # 8 q-chunks
NSPILL = 8             # ST k-tiles spilled to DRAM (rest stay resident)
F32R = mybir.dt.float32r
F32 = mybir.dt.float32
BF16 = mybir.dt.bfloat16

# E/UT production+consumption order: resident ST tiles (k=8..15) first.
KORDER = list(range(NSPILL, NK)) + list(range(NSPILL))

_NC_CACHE = {}


def _build():
    if "nc" in _NC_CACHE:
        return _NC_CACHE["nc"]
    nc = bacc.Bacc("TRN2", target_bir_lowering=False, debug=False,
                   num_devices=N_CORES)

    qT = nc.dram_tensor("qT", [D, SQ], F32R, kind="ExternalInput")
    kT = nc.dram_tensor("kT", [D, S], F32R, kind="ExternalInput")
    kraw = nc.dram_tensor("kraw", [S, D], BF16, kind="ExternalInput")
    wq = nc.dram_tensor("wq", [D, D], F32R, kind="ExternalInput")
    wk = nc.dram_tensor("wk", [D, D], F32R, kind="ExternalInput")
    wvT = nc.dram_tensor("wvT", [D, D], BF16, kind="ExternalInput")
    out = nc.dram_tensor("out", [SQ, D], F32, kind="ExternalOutput")

    from concourse.masks import make_identity

    with tile.TileContext(nc) as tc:
        with ExitStack() as ctx:
            psum = ctx.enter_context(tc.tile_pool(name="psum", bufs=6, space="PSUM"))
            psl = ctx.enter_context(tc.tile_pool(name="psl", bufs=1, space="PSUM"))
            dram = ctx.enter_context(tc.tile_pool(name="dram", bufs=1, space="DRAM"))
            consts = ctx.enter_context(tc.tile_pool(name="consts", bufs=1))
            tiny = ctx.enter_context(tc.tile_pool(name="tiny", bufs=1))
            pA = ctx.enter_context(tc.tile_pool(name="pA", bufs=16))
            pB = ctx.enter_context(tc.tile_pool(name="pB", bufs=8))
            pC = ctx.enter_context(tc.tile_pool(name="pC", bufs=8))
            pD = ctx.enter_context(tc.tile_pool(name="pD", bufs=8))
            pQ = ctx.enter_context(tc.tile_pool(name="pQ", bufs=8))

            id8 = consts.tile([8, 8], F32)
            make_identity(nc, id8[:])
            ones_c = consts.tile([P, 1], F32)
            nc.gpsimd.memset(ones_c[:], 1.0)

            st_spill = [dram.tile([P, SQ], F32, tag="stsp", name=f"stsp{i}")
                        for i in range(NSPILL)]

            dmae = [nc.sync, nc.scalar, nc.gpsimd]

            # ---- tile allocation in rotation order -----------------
            # wq/wk live as separate lo/hi column-half tiles (packed 2
            # per 4KB slot): wave-2 DMA writes then land in different
            # slots than the halves the P0 sweeps are reading, avoiding
            # SBUF read/write bank conflicts (measured ~6us of 300-430ns
            # matmuls when a single tile was half-read, half-written).
            wqlo = [pB.tile([P, D], F32R, tag="b", name=f"wqlo{j}")
                    for j in range(4)]
            wqhi = [pB.tile([P, D], F32R, tag="b", name=f"wqhi{j}")
                    for j in range(4)]
            wklo = [pC.tile([P, D], F32R, tag="c", name=f"wklo{j}")
                    for j in range(4)]
            wkhi = [pC.tile([P, D], F32R, tag="c", name=f"wkhi{j}")
                    for j in range(4)]

            def wq_ap(e, c0, w_):   # wq[e-tile][:, c0:c0+w_], c0 half-aligned
                src = wqlo if c0 < 512 else wqhi
                base = (e % 2) * 512 + (c0 % 512)
                return src[e // 2][:, base:base + w_]

            def wk_ap(e, c0, w_):
                src = wklo if c0 < 512 else wkhi
                base = (e % 2) * 512 + (c0 % 512)
                return src[e // 2][:, base:base + w_]
            qts = [pQ.tile([P, SQ], F32R, tag="q", name=f"qin{i}")
                   for i in range(ND)]
            ktsA = [pA.tile([P, SQ], F32R, tag="a", name=f"kta{i}")
                    for i in range(ND)]
            ktsB = [pA.tile([P, SQ], F32R, tag="a", name=f"ktb{i}")
                    for i in range(ND)]
            MT = [pD.tile([P, D], F32R, tag="d", name=f"mt{i}")
                  for i in range(ND)]

            # input loads in consumption order, round-robin 3 queues:
            # wq/wk pairs feed P0 immediately; qT lands during P0 for
            # P1; kT during P1 for P2. kraw/wv/reloads are issued later
            # (inside P2) once their victim slots are dead.
            for d in range(ND):
                dmae[(2 * d) % 3].dma_start(wq_ap(d, 0, 512),
                                            wq.ap()[ts(d, P), 0:512])
                dmae[(2 * d + 1) % 3].dma_start(wk_ap(d, 0, 512),
                                                wk.ap()[ts(d, P), 0:512])
            for d in range(ND):
                dmae[(2 * d) % 3].dma_start(wq_ap(d, 512, 512),
                                            wq.ap()[ts(d, P), 512:D])
                dmae[(2 * d + 1) % 3].dma_start(wk_ap(d, 512, 512),
                                                wk.ap()[ts(d, P), 512:D])
            for d in range(ND):
                dmae[d % 3].dma_start(qts[d][:], qT.ap()[ts(d, P), :])
            for d in range(ND):
                dmae[(2 * d) % 3].dma_start(ktsA[d][:], kT.ap()[ts(d, P), 0:SQ])
                dmae[(2 * d + 1) % 3].dma_start(ktsB[d][:], kT.ap()[ts(d, P), SQ:S])

            # ---- P0: MT[d,d'] = Wq^T Wk ----------------------------
            # e-inner over 4-d-groups: the first matmul needs only
            # wqs[0]+wks[0]; each arriving e-pair feeds 4 matmuls.
            for half in range(2):
                for dg in range(2):
                    pss = [psum.tile([P, 512], F32, tag="mm",
                                     name=f"ps_m{half}_{dg}_{i}")
                           for i in range(4)]
                    for e in range(NE):
                        for i in range(4):
                            nc.tensor.matmul(pss[i][:],
                                             wq_ap(e, (dg * 4 + i) * P, P),
                                             wk_ap(e, half * 512, 512),
                                             start=(e == 0),
                                             stop=(e == NE - 1))
                    for i in range(4):
                        nc.vector.tensor_copy(
                            MT[dg * 4 + i][:, ts(half, 512)], pss[i][:])

            # ---- P1: AT[d',q] = MT qT ------------------------------
            AT = [pC.tile([P, SQ], F32R, tag="c", name=f"at{i}")
                  for i in range(ND)]
            for qh in range(2):
                for dg in range(2):
                    pss = [psum.tile([P, 512], F32, tag="mm",
                                     name=f"ps_a{qh}_{dg}_{i}")
                           for i in range(4)]
                    for d in range(ND):
                        for i in range(4):
                            nc.tensor.matmul(pss[i][:],
                                             MT[d][:, ts(dg * 4 + i, P)],
                                             qts[d][:, ts(qh, 512)],
                                             start=(d == 0),
                                             stop=(d == ND - 1))
                    for i in range(4):
                        nc.vector.tensor_copy(
                            AT[dg * 4 + i][:, ts(qh, 512)], pss[i][:])

            # softmax scratch lands in B (wq victims, dead after P0)
            m_bc = pB.tile([P, SQ], F32, tag="b", name="m_bc")
            macc = pB.tile([P, SQ], F32, tag="b", name="macc")
            lacc = pB.tile([P, SQ], F32, tag="b", name="lacc")
            spare = pB.tile([P, SQ], F32, tag="b", name="spare")  # noqa: F841

            # ---- P2: ST[k,q] = kT^T AT; DVE row-max on the fly -----
            # k=0..7 spill to DRAM (slot reused 8 tiles later by the
            # rotation); k=8..15 stay resident in D (MT victims).
            st_tiles = {}
            krs2 = []

            def st_chain(st_k, k, qh):
                # one (k, qh) score chain + drain + per-half running
                # max; each half's partition reduce launches the moment
                # the last chain for that half drains. The final two k
                # iterations are emitted half-interleaved (qh0 chains
                # of k=14,15 first) so the qh0 reduce hides entirely
                # under the remaining qh1 chains.
                sl = ts(qh, 512)
                kts = ktsA if k < 8 else ktsB
                ps = psum.tile([P, 512], F32, tag="mm", name=f"ps_s{k}_{qh}")
                for dp in range(ND):
                    nc.tensor.matmul(ps[:], kts[dp][:, ts(k % 8, P)],
                                     AT[dp][:, ts(qh, 512)],
                                     start=(dp == 0), stop=(dp == ND - 1))
                nc.vector.tensor_copy(st_k[:, sl], ps[:])
                if k == 0:
                    nc.vector.tensor_copy(macc[:, sl], st_k[:, sl])
                else:
                    nc.vector.tensor_max(macc[:, sl], macc[:, sl],
                                         st_k[:, sl])
                if k == NK - 1:
                    nc.gpsimd.partition_all_reduce(
                        m_bc[:, sl], macc[:, sl], channels=P,
                        reduce_op=bass_isa.ReduceOp.max)

            for k in range(NK - 2):
                st_k = pD.tile([P, SQ], F32, tag="d", name=f"stb{k}")
                for qh in range(2):
                    st_chain(st_k, k, qh)
                if k < NSPILL:
                    nc.sync.dma_start(st_spill[k][:], st_k[:])
                else:
                    st_tiles[k] = st_k
                if k == 7:
                    # ktsA is dead: issue kraw (its victim) now so the
                    # 4MB lands under P2's second half; ST reloads (qts
                    # victims, dead since P1) prefetch on gpsimd.
                    krs2 = [pA.tile([P, 2 * SQ], BF16, tag="a",
                                    name=f"kr{j}") for j in range(NK // 2)]
                    for j in range(NK // 2):
                        for h in range(2):
                            kk2 = KORDER[2 * j + h]
                            dmae[(2 * j + h) % 2].dma_start(
                                krs2[j][:, ts(h, SQ)],
                                kraw.ap()[ts(kk2, P), :])
                    for kr in range(NSPILL):
                        st_r = pQ.tile([P, SQ], F32, tag="q",
                                       name=f"rl{kr}")
                        nc.gpsimd.dma_start(st_r[:], st_spill[kr][:])
                        st_tiles[kr] = st_r
            st14 = pD.tile([P, SQ], F32, tag="d", name="stb14")
            st15 = pD.tile([P, SQ], F32, tag="d", name="stb15")
            st_tiles[NK - 2], st_tiles[NK - 1] = st14, st15
            st_chain(st14, NK - 2, 0)
            st_chain(st15, NK - 1, 0)
            st_chain(st14, NK - 2, 1)
            st_chain(st15, NK - 1, 1)

            # wv (AT victims, dead at P2 end) + UT slots in C
            wv2 = [pC.tile([P, 2 * SQ], BF16, tag="c", name=f"wv{j}")
                   for j in range(ND // 2)]
            for j in range(ND // 2):
                for h in range(2):
                    nc.sync.dma_start(
                        wv2[j][:, ts(h, SQ)], wvT.ap()[ts(2 * j + h, P), :])
            ut2 = [pC.tile([P, 2 * SQ], BF16, tag="c", name=f"ut{j}")
                   for j in range(ND // 2)]

            # ---- P4+P5 interleaved: E = exp(ST - m) bf16, l on DVE;
            # UT[d,q] = key^T E, 4 concurrent PSUM groups, k-outer ----
            E2 = [pA.tile([P, 2 * SQ], BF16, tag="a", name=f"e{j}")
                  for j in range(NK // 2)]

            def e_ap(i, qh):        # [P,512] E view for KORDER[i]
                return E2[i // 2][:, (i % 2) * SQ + qh * 512:
                                  (i % 2) * SQ + qh * 512 + 512]

            def kr_ap(i, dp):       # lhsT slice for KORDER[i], d-tile dp
                return krs2[i // 2][:, (i % 2) * SQ + dp * P:
                                    (i % 2) * SQ + (dp + 1) * P]

            def ut_ap(dp, c0, w_):  # [P,w_] UT view for d-tile dp
                return ut2[dp // 2][:, (dp % 2) * SQ + c0:
                                    (dp % 2) * SQ + c0 + w_]

            for qh in range(2):
                sl = ts(qh, 512)
                for i, k in enumerate(KORDER):
                    st_k = st_tiles[k]
                    nc.vector.tensor_sub(st_k[:, sl], st_k[:, sl],
                                         m_bc[:, sl])
                    nc.scalar.activation(e_ap(i, qh), st_k[:, sl],
                                         mybir.ActivationFunctionType.Exp)
                    if i == 1:
                        nc.vector.tensor_add(lacc[:, sl], e_ap(0, qh),
                                             e_ap(1, qh))
                    elif i > 1:
                        nc.vector.tensor_add(lacc[:, sl], lacc[:, sl],
                                             e_ap(i, qh))
                # 6-group then 2-group sweeps: the wider first sweep
                # consumes each E tile 6x (1.4us) vs the 0.7us exp
                # cadence, so the PE rides out the exp-chain warmup.
                for d0, gw in ((0, 6), (6, 2)):
                    pss = [psum.tile([P, 512], F32, tag="mm",
                                     name=f"ps_u{qh}_{d0}_{i}")
                           for i in range(gw)]
                    for i in range(NK):
                        for t in range(gw):
                            nc.tensor.matmul(pss[t][:],
                                             kr_ap(i, d0 + t),
                                             e_ap(i, qh),
                                             start=(i == 0),
                                             stop=(i == NK - 1))
                    for t in range(gw):
                        nc.vector.tensor_copy(
                            ut_ap(d0 + t, qh * 512, 512), pss[t][:])

            # ---- P6: O[q,e] = UT^T Wv^T; 1/l path threaded in ------
            groups = [(qc, eh) for qc in range(NQC) for eh in range(D // 512)]
            l_row = tiny.tile([1, SQ], F32)
            r_dram = dram.tile([1, SQ], F32)
            r8 = tiny.tile([8, P], F32)
            pt8 = psl.tile([P, 8], F32, tag="pt8")
            recip_t = tiny.tile([P, 8], F32)
            pending = []

            def emit_store(qc, eh, ot, i):
                nc.vector.tensor_scalar_mul(ot[:], ot[:], recip_t[:, qc:qc + 1])
                eng = nc.sync if i % 2 == 0 else nc.scalar
                eng.dma_start(out.ap()[ts(qc, P), ts(eh, 512)], ot[:])

            def wv_ap(dp, eh):      # [P,512] Wv^T view for d-tile dp
                return wv2[dp // 2][:, (dp % 2) * SQ + eh * 512:
                                    (dp % 2) * SQ + eh * 512 + 512]

            for g, (qc, eh) in enumerate(groups):
                ps = psum.tile([P, 512], F32, tag="mm", name=f"ps_o{qc}_{eh}")
                for dp in range(ND):
                    nc.tensor.matmul(ps[:], ut_ap(dp, qc * P, P),
                                     wv_ap(dp, eh),
                                     start=(dp == 0), stop=(dp == ND - 1))
                ot = pB.tile([P, 512], F32, tag="b", name=f"ot{qc}_{eh}")
                nc.vector.tensor_copy(ot[:], ps[:])
                if g < 5:
                    pending.append((qc, eh, ot))
                else:
                    emit_store(qc, eh, ot, g)
                if g == 2:
                    for qh in range(2):
                        pl = psl.tile([1, 512], F32, tag="pl", name=f"pl{qh}")
                        nc.tensor.matmul(pl[:], ones_c[:], lacc[:, ts(qh, 512)],
                                         start=True, stop=True)
                        nc.vector.tensor_copy(l_row[0:1, ts(qh, 512)], pl[:])
                    nc.sync.dma_start(r_dram[:], l_row[:])
                    nc.sync.dma_start(
                        r8[:], r_dram[0, :].rearrange("(a b) -> a b", a=8))
                elif g == 4:
                    nc.tensor.transpose(pt8[:], r8[:], id8[:])
                    nc.vector.reciprocal(recip_t[:], pt8[:])
                    for i, (pqc, peh, pot) in enumerate(pending):
                        emit_store(pqc, peh, pot, i)

    nc.compile()
    _NC_CACHE["nc"] = nc
    return nc


def make_in_maps(query, key, Wq, Wk, Wv):
    query = np.asarray(query, dtype=np.float32)
    key = np.asarray(key, dtype=np.float32)
    wq_np = np.ascontiguousarray(np.asarray(Wq, dtype=np.float32))
    wk_np = np.ascontiguousarray(np.asarray(Wk, dtype=np.float32))
    wvT_np = np.ascontiguousarray(
        np.asarray(Wv, dtype=np.float32).T.astype(ml_dtypes.bfloat16))

    in_maps = []
    for c in range(N_CORES):
        b, h = c // 2, c % 2
        qTn = np.ascontiguousarray(query[b, h * SQ:(h + 1) * SQ, :].T)
        kTn = np.ascontiguousarray(key[b].T)
        krn = np.ascontiguousarray(key[b].astype(ml_dtypes.bfloat16))
        in_maps.append({
            "qT": qTn, "kT": kTn, "kraw": krn,
            "wq": wq_np, "wk": wk_np, "wvT": wvT_np,
        })
    return in_maps


def kernel(query, key, Wq, Wk, Wv):
    in_maps = make_in_maps(query, key, Wq, Wk, Wv)
    nc = _build()
    res = run_bass_kernel_spmd(nc, in_maps, core_ids=list(range(N_CORES)))
    outv = np.empty((B, S, D), dtype=np.float32)
    for c in range(N_CORES):
        b, h = c // 2, c % 2
        outv[b, h * SQ:(h + 1) * SQ, :] = res.results[c]["out"]
    return outv
